# revision 1
# baseline (speedup 1.0000x reference)
"""Trainium2 Bass kernel for a heterogeneous GraphConv layer (3 relations).

out = concat([leaky(GC(inst_feat, W_inst, in_*)),     # -> node   (10000)
              leaky(GC(node_feat, W_node, ni_*)),     # -> inst   (100000)
              leaky(GC(svc_feat,  W_svc,  sc_*))])    # -> svc    (20000)

GC(f, W, src, dst) = rsqrt(deg_d) * segsum_dst((rsqrt(deg_s)*f)[src]) @ W + b
(aggregation commutes with the dense @W, so we gather *raw scaled features*
and apply W once per destination tile).

Strategy: destination-sharded across 8 NeuronCores. Per core and relation the
edges (sorted by dst) are grouped into 128-edge blocks per 128-dst tile.
Device work per block: dma_gather 128 source rows from DRAM -> [128e,128f],
DVE builds a one-hot S^T[e,dst]=(iota==dst_local) tile, PE accumulates
aggT[f,dst] += G.T @ S^T in PSUM.  Per dst tile: PSUM out = u (x) b (rank-1
K=1 matmul preloading bias/rsqrt_deg_d) + aggT.T @ W, then one ScalarE
Lrelu(out * rsqrt_deg_d) and a DMA to the output rows.
"""

import os as _os

import numpy as np

SVC_N, INST_N, NODE_N, HID = 20000, 100000, 10000, 128
NCORES = 8
BLK = 128           # edges per one-hot matmul block
# blocks per dma_gather instruction. NOTE: needs single_packet=False — with
# single_packet=True the whole stream coalesces into one DMA packet, which
# caps at 64 descriptors/engine (num_idxs <= 1024); beyond that the exec
# unit faults (NRT_EXEC_UNIT_UNRECOVERABLE).
CHUNK = int(_os.environ.get("GNN_CHUNK", "32"))
# one-hot builder batching: 1 = tensor_scalar per block; >1 = one DVE
# tensor_tensor(is_equal) per EQG consecutive blocks (dl broadcast stride-0)
EQG = int(_os.environ.get("GNN_EQG", "1"))
# gather/one-hot dtype: fp16 (default) has a 10-bit mantissa (~4x better
# than bf16), runs the PE at 1 cyc/row like bf16, and halves gather bytes
# vs fp32. GNN_BF16=1 kept for A/B.
GDT = _os.environ.get("GNN_DT", "fp16")
if int(_os.environ.get("GNN_BF16", "0")):
    GDT = "bf16"
# split mode: tables hold [hi|lo] half-precision pairs (512B rows); two
# accumulating matmuls per block recover ~fp32 precision at 2x PE cost
USE_SPLIT = bool(int(_os.environ.get("GNN_SPLIT", "0")))
if USE_SPLIT and GDT == "fp32":
    GDT = "fp16"
USE_BF16 = GDT != "fp32"  # legacy name: True when gather dtype is 16-bit
ACT_MODE = "lrelu"  # "lrelu" (HW leaky relu) | "relu" (sim debug)

_cache = {}


def _cdiv(a, b):
    return (a + b - 1) // b


def _rup(a, b):
    return _cdiv(a, b) * b


def _gdt_np():
    if GDT == "fp32":
        return np.dtype(np.float32)
    if GDT == "fp16":
        return np.dtype(np.float16)
    import ml_dtypes
    return np.dtype(ml_dtypes.bfloat16)


def _prep_relation(src, dst, n_src, n_dst, feat, compact):
    """Host-side sharding/packing for one relation."""
    src = np.asarray(src, np.int64)
    dst = np.asarray(dst, np.int64)
    deg_s = np.maximum(np.bincount(src, minlength=n_src), 1).astype(np.float64)
    deg_d = np.maximum(np.bincount(dst, minlength=n_dst), 1).astype(np.float64)
    rs_s = (1.0 / np.sqrt(deg_s)).astype(np.float32)
    rs_d = (1.0 / np.sqrt(deg_d)).astype(np.float32)
    u_d = np.sqrt(deg_d).astype(np.float32)  # ~= 1/rs_d

    feat_s = (np.asarray(feat, np.float32) * rs_s[:, None]).astype(np.float32)

    D = _rup(_cdiv(n_dst, NCORES), 128)  # dst rows per core (padded)
    ntiles = D // 128

    cores = []
    counts = np.zeros((NCORES, ntiles), np.int64)
    for c in range(NCORES):
        lo, hi = c * D, (c + 1) * D
        m = (dst >= lo) & (dst < hi)
        es, ed = src[m], dst[m] - lo
        order = np.argsort(ed, kind="stable")
        es, ed = es[order], ed[order]
        if compact:
            uniq, inv = np.unique(es, return_inverse=True)
            gidx = inv.astype(np.int64)
            table = feat_s[uniq]
        else:
            gidx = es
            table = None
        tile_of = ed >> 7
        counts[c] = np.bincount(tile_of, minlength=ntiles)
        cores.append(dict(gidx=gidx, dloc=(ed & 127).astype(np.float32),
                          table=table, tile_counts=counts[c]))

    # blocks per tile: max over cores, >=1 so PSUM always gets a start matmul
    btile = np.maximum(_cdiv(np.max(counts, axis=0), BLK), 1).astype(np.int64)
    nblk = int(btile.sum())
    nblk_pad = _rup(nblk, CHUNK)

    for c in range(NCORES):
        d = cores[c]
        g = np.zeros(nblk_pad * BLK, np.int64)
        dl = np.full(nblk_pad * BLK, -1.0, np.float32)
        pos = 0  # edge cursor in sorted arrays
        off = 0  # block-slot cursor
        for t in range(ntiles):
            n = int(d["tile_counts"][t])
            g[off:off + n] = d["gidx"][pos:pos + n]
            dl[off:off + n] = d["dloc"][pos:pos + n]
            pos += n
            off += int(btile[t]) * BLK
        # relation-tail pad slots: idx -1 -> trailing negatives are skipped
        # by the gather engine (num_idxs_reg trimmed per chunk device-side)
        g[nblk * BLK:] = -1
        d["gflat"], d["dlflat"] = g, dl
        del d["gidx"], d["dloc"]

    return dict(cores=cores, btile=btile, nblk=nblk, nblk_pad=nblk_pad,
                ntiles=ntiles, D=D, feat_s=feat_s, rs_d=rs_d, u_d=u_d,
                n_dst=n_dst, compact=compact)


def _build_host_data(inputs):
    rels = [
        # order matters: output rows are [node_out, inst_out, svc_out]
        _prep_relation(inputs["in_src"], inputs["in_dst"], INST_N, NODE_N,
                       inputs["instance_feat"], compact=True),
        _prep_relation(inputs["ni_src"], inputs["ni_dst"], NODE_N, INST_N,
                       inputs["node_feat"], compact=False),
        _prep_relation(inputs["sc_src"], inputs["sc_dst"], SVC_N, SVC_N,
                       inputs["svc_feat"], compact=False),
    ]
    Ws = [inputs["W_inst"], inputs["W_node"], inputs["W_svc"]]
    bs = [inputs["b_inst"], inputs["b_node"], inputs["b_svc"]]

    gdt = _gdt_np()

    umax = _rup(max(len(rels[0]["cores"][c]["table"]) for c in range(NCORES)), 16)
    nblk_tot = sum(r["nblk_pad"] for r in rels)
    nidx_tot = nblk_tot * BLK
    ntile_tot = sum(r["ntiles"] for r in rels)

    W_cat = np.concatenate([np.asarray(w, np.float32) for w in Ws], axis=1)
    b_row = np.concatenate([np.asarray(b, np.float32) for b in bs])[None, :]
    iota = np.tile(np.arange(128, dtype=np.float32), (128, max(1, EQG))).astype(gdt)

    in_maps = []
    for c in range(NCORES):
        gidx = np.concatenate([r["cores"][c]["gflat"] for r in rels])
        dl = np.concatenate([r["cores"][c]["dlflat"] for r in rels])
        assert gidx.max() < 32768, "gather idx must fit int16"
        # gather layout: idx i -> partition i%16, col i//16, replicated x8
        idx16 = np.ascontiguousarray(gidx.astype(np.int16).reshape(-1, 16).T)
        idx_sb = np.tile(idx16, (8, 1))                          # [128, nidx/16]
        dl_sb = np.ascontiguousarray(
            dl.reshape(nblk_tot, BLK).T).astype(np.float32)      # [128, nblk]

        rs_sb = np.zeros((128, ntile_tot), np.float32)
        u_sb = np.zeros((1, ntile_tot * 128), np.float32)
        t0 = 0
        for r in rels:
            lo = c * r["D"]
            val_rs = np.zeros(r["D"], np.float32)
            val_u = np.zeros(r["D"], np.float32)
            n = max(0, min(r["D"], r["n_dst"] - lo))
            if n > 0:
                val_rs[:n] = r["rs_d"][lo:lo + n]
                val_u[:n] = r["u_d"][lo:lo + n]
            rs_sb[:, t0:t0 + r["ntiles"]] = val_rs.reshape(r["ntiles"], 128).T
            u_sb[0, t0 * 128:(t0 + r["ntiles"]) * 128] = val_u
            t0 += r["ntiles"]

        tbl_in = np.zeros((umax, HID), np.float32)
        t = rels[0]["cores"][c]["table"]
        tbl_in[:len(t)] = t

        def _tbl(x):
            hi = x.astype(gdt)
            if not USE_SPLIT:
                return np.ascontiguousarray(hi)
            lo = (x - hi.astype(np.float32)).astype(gdt)
            return np.ascontiguousarray(np.concatenate([hi, lo], axis=1))

        in_maps.append({
            "tbl_in": _tbl(tbl_in),
            "tbl_ni": _tbl(rels[1]["feat_s"]),
            "tbl_sc": _tbl(rels[2]["feat_s"]),
            "idx_sb": np.ascontiguousarray(idx_sb),
            "dl_sb": dl_sb,
            "rs_sb": rs_sb,
            "u_sb": u_sb,
            "W_cat": np.ascontiguousarray(W_cat),
            "b_row": np.ascontiguousarray(b_row),
            "iota": np.ascontiguousarray(iota),
        })

    meta = dict(
        umax=umax, nblk_tot=nblk_tot, nidx_tot=nidx_tot, ntile_tot=ntile_tot,
        btiles=[r["btile"].tolist() for r in rels],
        ntiles=[r["ntiles"] for r in rels],
        Ds=[r["D"] for r in rels],
        n_dsts=[r["n_dst"] for r in rels],
    )
    return meta, in_maps


def _build_program(meta):
    import os

    import concourse.bacc as bacc
    import concourse.mybir as mybir
    import concourse.tile as tile

    dbg_max_tiles = int(os.environ.get("GNN_MAX_TILES", "0"))  # 0 = all
    dbg_skip_gather = bool(int(os.environ.get("GNN_SKIP_GATHER", "0")))
    assert CHUNK % EQG == 0, "eq-groups must align with gather chunks"
    # dst tiles batched per epilogue staging buffer / out DMA
    OUT_GRP = int(os.environ.get("GNN_OUT_GRP", "4"))

    gdt = {"fp32": mybir.dt.float32, "fp16": mybir.dt.float16,
           "bf16": mybir.dt.bfloat16}[GDT]
    f32 = mybir.dt.float32
    AF = mybir.ActivationFunctionType
    act_fn = AF.Lrelu if ACT_MODE == "lrelu" else AF.Relu

    nblk_tot, nidx_tot, ntile_tot = meta["nblk_tot"], meta["nidx_tot"], meta["ntile_tot"]
    umax = meta["umax"]

    nc = bacc.Bacc("TRN2", target_bir_lowering=False, debug=False,
                   enable_asserts=False, num_devices=NCORES)

    TW = 2 * HID if USE_SPLIT else HID  # table row width
    tbl_d = [
        nc.dram_tensor("tbl_in", [umax, TW], gdt, kind="ExternalInput"),
        nc.dram_tensor("tbl_ni", [NODE_N, TW], gdt, kind="ExternalInput"),
        nc.dram_tensor("tbl_sc", [SVC_N, TW], gdt, kind="ExternalInput"),
    ]
    idx_d = nc.dram_tensor("idx_sb", [128, nidx_tot // 16], mybir.dt.int16,
                           kind="ExternalInput")
    dl_d = nc.dram_tensor("dl_sb", [128, nblk_tot], f32, kind="ExternalInput")
    rs_d = nc.dram_tensor("rs_sb", [128, ntile_tot], f32, kind="ExternalInput")
    u_d = nc.dram_tensor("u_sb", [1, ntile_tot * 128], f32, kind="ExternalInput")
    W_d = nc.dram_tensor("W_cat", [128, 3 * HID], f32, kind="ExternalInput")
    b_d = nc.dram_tensor("b_row", [1, 3 * HID], f32, kind="ExternalInput")
    iota_d = nc.dram_tensor("iota", [128, EQG * 128], gdt, kind="ExternalInput")

    out_d = [
        nc.dram_tensor("out_node", [meta["Ds"][0], HID], f32, kind="ExternalOutput"),
        nc.dram_tensor("out_inst", [meta["Ds"][1], HID], f32, kind="ExternalOutput"),
        nc.dram_tensor("out_svc", [meta["Ds"][2], HID], f32, kind="ExternalOutput"),
    ]

    with tile.TileContext(nc) as tc:
        with (
            tc.tile_pool(name="const", bufs=1) as const,
            tc.tile_pool(name="g", bufs=6) as gpool,
            tc.tile_pool(name="st", bufs=8) as stpool,
            tc.tile_pool(name="evac", bufs=4) as evac,
            tc.tile_pool(name="osb", bufs=6) as opool,
            tc.tile_pool(name="psA", bufs=4, space="PSUM") as psA,
            tc.tile_pool(name="psO", bufs=3, space="PSUM") as psO,
        ):
            # first gather depends only on the leading idx slice + dl/iota:
            # load those first so the gather stream starts ASAP (HWDGE is
            # FIFO per issuing engine)
            idx_t = const.tile([128, nidx_tot // 16], mybir.dt.int16)
            c0 = min(4 * CHUNK * BLK // 16, nidx_tot // 16)
            nc.sync.dma_start(idx_t[:, :c0], idx_d.ap()[:, :c0])
            dl_t = const.tile([128, nblk_tot], f32)
            nc.sync.dma_start(dl_t[:], dl_d.ap())
            iota_t = const.tile([128, EQG * 128], gdt)
            nc.sync.dma_start(iota_t[:], iota_d.ap())
            W_t = const.tile([128, 3 * HID], f32)
            nc.sync.dma_start(W_t[:], W_d.ap())
            b_t = const.tile([1, 3 * HID], f32)
            nc.sync.dma_start(b_t[:], b_d.ap())
            u_t = const.tile([1, ntile_tot * 128], f32)
            nc.sync.dma_start(u_t[:], u_d.ap())
            rs_t = const.tile([128, ntile_tot], f32)
            nc.sync.dma_start(rs_t[:], rs_d.ap())
            if c0 < nidx_tot // 16:
                nc.sync.dma_start(idx_t[:, c0:], idx_d.ap()[:, c0:])

            g_tiles = {}   # chunk -> gather tile
            st_tiles = {}  # eq-group -> one-hot tile [128, EQG, 128]

            def issue_st(gi):
                g0 = gi * EQG
                gsz = min(EQG, nblk_tot - g0)
                st = stpool.tile([128, EQG, 128], gdt, tag="st")
                if EQG == 1:
                    nc.vector.tensor_scalar(
                        st[:, 0, :], iota_t[:, 0:128], dl_t[:, g0:g0 + 1], None,
                        mybir.AluOpType.is_equal)
                else:
                    nc.vector.tensor_tensor(
                        st[:, :gsz, :],
                        iota_t[:, :gsz * 128].rearrange("p (g k) -> p g k", k=128),
                        dl_t[:, g0:g0 + gsz].broadcast_to([128, gsz, 128]),
                        mybir.AluOpType.is_equal)
                st_tiles[gi] = st

            def issue_gather(ci, rel, rel_blk0, rel_nblk):
                gt = gpool.tile([128, CHUNK, TW], gdt, tag="g")
                nidx = CHUNK * BLK
                # trailing -1 idxs (relation-tail pads) are skipped; trim reg
                local0 = ci * CHUNK - rel_blk0
                real_blocks = max(0, min(CHUNK, rel_nblk - local0))
                if dbg_skip_gather:
                    nc.vector.memset(gt[:], 1.0)
                else:
                    nc.gpsimd.dma_gather(
                        out_ap=gt[:],
                        in_ap=tbl_d[rel].ap(),
                        idxs_ap=idx_t[:, ci * (nidx // 16):(ci + 1) * (nidx // 16)],
                        num_idxs=nidx,
                        num_idxs_reg=max(BLK, real_blocks * BLK),
                        elem_size=TW,
                        single_packet=False,
                    )
                g_tiles[ci] = gt

            blk = 0   # global block cursor
            tglob = 0  # global tile cursor
            for rel in range(3):
                ntiles = meta["ntiles"][rel]
                btile = meta["btiles"][rel]
                rel_base = blk
                rel_nblk = sum(btile)
                for t in range(ntiles):
                    if dbg_max_tiles and t >= dbg_max_tiles:
                        break
                    nb = btile[t]
                    agg = psA.tile([128, 128], f32, tag="agg")
                    for b in range(nb):
                        ci, cj = divmod(blk, CHUNK)
                        if cj == 0:
                            issue_gather(ci, rel, rel_base, rel_nblk)
                        gi, gj = divmod(blk, EQG)
                        if gj == 0:
                            issue_st(gi)
                        for part in range(2 if USE_SPLIT else 1):
                            nc.tensor.matmul(
                                agg[:],
                                g_tiles[ci][:, cj, part * HID:(part + 1) * HID],
                                st_tiles[gi][:, gj, :],
                                start=(b == 0 and part == 0),
                                stop=(b == nb - 1 and part == (1 if USE_SPLIT else 0)))
                        blk += 1
                    # pad blocks at relation tail are gathered but never used
                    aggsb = evac.tile([128, 128], f32, tag="evac")
                    nc.scalar.copy(aggsb[:], agg[:])
                    po = psO.tile([128, 128], f32, tag="po")
                    nc.tensor.matmul(
                        po[:], u_t[:, tglob * 128:(tglob + 1) * 128],
                        b_t[:, rel * HID:(rel + 1) * HID],
                        start=True, stop=False, skip_group_check=True)
                    nc.tensor.matmul(
                        po[:], aggsb[:], W_t[:, rel * HID:(rel + 1) * HID],
                        start=False, stop=True, skip_group_check=True)
                    oj = t % OUT_GRP
                    if oj == 0:
                        osb = opool.tile([128, OUT_GRP, 128], f32, tag="osb")
                        osb_t0 = t
                    nc.scalar.activation(
                        osb[:, oj, :], po[:], act_fn,
                        bias=0.0, scale=rs_t[:, tglob:tglob + 1], alpha=0.01)
                    if oj == OUT_GRP - 1 or t == ntiles - 1:
                        cnt = t - osb_t0 + 1
                        dst = out_d[rel].ap()[osb_t0 * 128:(t + 1) * 128, :]
                        nc.sync.dma_start(
                            dst.rearrange("(j p) k -> p j k", p=128),
                            osb[:, :cnt, :])
                    tglob += 1
                # advance block cursor past relation tail padding
                rel_end_pad = (-blk) % CHUNK
                blk += rel_end_pad

    nc.compile()
    return nc


def _run(nc, in_maps, trace=False, **kw):
    from concourse import bass_utils
    res = bass_utils.run_bass_kernel_spmd(
        nc, in_maps, core_ids=list(range(NCORES)), trace=trace, **kw)
    return res


def _assemble(results, meta):
    out = np.empty((NODE_N + INST_N + SVC_N, HID), np.float32)
    offs = [0, NODE_N, NODE_N + INST_N]
    names = ["out_node", "out_inst", "out_svc"]
    for rel in range(3):
        D, n_dst = meta["Ds"][rel], meta["n_dsts"][rel]
        for c in range(NCORES):
            lo = c * D
            n = max(0, min(D, n_dst - lo))
            if n > 0:
                out[offs[rel] + lo: offs[rel] + lo + n] = \
                    results[c][names[rel]][:n]
    return out


def kernel(**inputs):
    import hashlib
    key = "prog"
    h = hashlib.sha1()
    for k in ("sc_src", "sc_dst", "in_src", "in_dst", "ni_src", "ni_dst"):
        h.update(np.ascontiguousarray(np.asarray(inputs[k], np.int32)).tobytes())
    sig = h.hexdigest()
    meta, in_maps = _build_host_data(inputs)
    if key in _cache and _cache[key][0] == sig:
        _, nc, _ = _cache[key]
    else:
        nc = _build_program(meta)
        _cache[key] = (sig, nc, meta)
    res = _run(nc, in_maps)
    return _assemble(res.results, meta)



# revision 7
# speedup vs baseline: 1.1922x; 1.1922x over previous
"""Trainium2 Bass kernel for a heterogeneous GraphConv layer (3 relations).

out = concat([leaky(GC(inst_feat, W_inst, in_*)),     # -> node   (10000)
              leaky(GC(node_feat, W_node, ni_*)),     # -> inst   (100000)
              leaky(GC(svc_feat,  W_svc,  sc_*))])    # -> svc    (20000)

GC(f, W, src, dst) = rsqrt(deg_d) * segsum_dst((rsqrt(deg_s)*f)[src]) @ W + b
(aggregation commutes with the dense @W, so we gather *raw scaled features*
and apply W once per destination tile).

Strategy: destination-sharded across 8 NeuronCores.  The per-core source
tables are PERMUTED so that rows co-used by the same dst tile sit adjacently;
each dma_gather descriptor then uses an overlapping 512B window (elem 256
fp16 elems, step 128) that fetches TWO consecutive rows — one descriptor
serves up to two edges (lanes A/B).  Descriptor cost on TRN2 is identical
for 256B and 512B payloads, so pairing halves gather DMA time.

Edges (sorted by dst) are packed densely into 128-slot blocks with per-tile
slot quotas (max over cores) so the block->tile map is identical on every
core; blocks straddling a dst-tile boundary contribute to 2+ tiles via
shifted one-hots (iota + 128k).  Per block and lane: DVE builds a one-hot
S[slot,dst]=(dl==iota_k), PE accumulates agg[f,dst] += G_lane.T @ S in PSUM.
Per dst tile: PSUM out = u (x) b (rank-1 bias preload) + agg.T @ W, one
ScalarE Lrelu(out * rsqrt_deg_d), fp16 transposed output DMA (the host
de-transposes and converts).
"""

import os as _os
from collections import defaultdict

import numpy as np

SVC_N, INST_N, NODE_N, HID = 20000, 100000, 10000, 128
NCORES = 8
BLK = 128           # slots per block (= PE contraction dim)
LANES = 2           # table rows per gather window (512B / 256B fp16 rows)
CHUNK = int(_os.environ.get("GNN_CHUNK", "32"))   # blocks per gather instr
EQG = int(_os.environ.get("GNN_EQG", "4"))        # blocks per batched one-hot
OUT_GRP = int(_os.environ.get("GNN_OUT_GRP", "8"))  # dst tiles per out DMA
ACT_MODE = "lrelu"

_cache = {}


def _cdiv(a, b):
    return (a + b - 1) // b


def _rup(a, b):
    return _cdiv(a, b) * b


def _sequence_sources(es, tile):
    """Order this core's used sources so same-tileset sources are adjacent.

    es/tile: per-edge (sorted by (src, tile)).  Returns (seq_srcs, starts,
    ends) where seq_srcs is the source order and starts/ends delimit each
    source's edge range in the (src,tile)-sorted arrays.
    """
    n = len(es)
    starts = np.flatnonzero(np.r_[True, es[1:] != es[:-1]])
    ends = np.r_[starts[1:], n]
    keys = [tuple(tile[a:b]) for a, b in zip(starts, ends)]
    order = sorted(range(len(starts)), key=lambda i: keys[i])
    return order, starts, ends


def _prep_relation(src, dst, n_src, n_dst, feat_s, compact):
    """Host-side sharding/packing for one relation.

    Returns dict with per-core slot streams and shared structure info.
    """
    src = np.asarray(src, np.int64)
    dst = np.asarray(dst, np.int64)

    D = _rup(_cdiv(n_dst, NCORES), 128)  # dst rows per core (padded)
    ntiles = D // 128

    cores = []
    for c in range(NCORES):
        lo = c * D
        m = (dst >= lo) & (dst < lo + D)
        es, ed = src[m], dst[m] - lo
        tl = ed >> 7
        order = np.lexsort((tl, es))
        es, ed, tl = es[order], ed[order], tl[order]

        uorder, starts, ends = _sequence_sources(es, tl)
        srcs_u = es[starts]
        nsrc_u = len(srcs_u)

        # position of each unique source in the permuted table
        pos_of_u = np.empty(nsrc_u, np.int64)
        pos_of_u[uorder] = np.arange(nsrc_u)

        if compact:
            table = feat_s[srcs_u[uorder]]
            n_units = nsrc_u
        else:
            # full table permuted: used sources in sequence order first,
            # then unused rows
            used_mask = np.zeros(n_src, bool)
            used_mask[srcs_u] = True
            perm = np.concatenate([srcs_u[uorder],
                                   np.flatnonzero(~used_mask)])
            table = feat_s[perm]
            n_units = n_src

        # per tile: list of (position, dst_local) per edge
        # build slots tile by tile with the path-greedy pairing
        slot_k = [[] for _ in range(ntiles)]
        slot_dA = [[] for _ in range(ntiles)]
        slot_dB = [[] for _ in range(ntiles)]
        # gather per-(tile) positions: iterate unique srcs, distribute edges
        per_tile = defaultdict(list)  # tile -> list of (pos, [dst_locals])
        for ui in range(nsrc_u):
            a, b = starts[ui], ends[ui]
            p = pos_of_u[ui]
            # edges of this src grouped by tile (tl sorted within src range)
            t0 = a
            while t0 < b:
                t1 = t0
                while t1 < b and tl[t1] == tl[t0]:
                    t1 += 1
                per_tile[tl[t0]].append((p, ed[t0:t1]))
                t0 = t1
        for t, lst in per_tile.items():
            lst.sort(key=lambda x: x[0])
            sk, sa, sb = slot_k[t], slot_dA[t], slot_dB[t]
            prev_pos = -10
            prev_ds = []  # unpaired dst_locals of prev position
            for p, ds in lst:
                ds = list(ds)
                if p == prev_pos + 1 and prev_ds:
                    npair = min(len(prev_ds), len(ds))
                    for i in range(npair):
                        sk.append(prev_pos)
                        sa.append(prev_ds[i])
                        sb.append(ds[i])
                    for d in prev_ds[npair:]:
                        sk.append(prev_pos)
                        sa.append(d)
                        sb.append(-1)
                    ds = ds[npair:]
                else:
                    for d in prev_ds:
                        sk.append(prev_pos)
                        sa.append(d)
                        sb.append(-1)
                prev_pos, prev_ds = p, ds
            for d in prev_ds:
                sk.append(prev_pos)
                sa.append(d)
                sb.append(-1)
            # put paired slots first so lane-B tails can be skipped
            osort = sorted(range(len(sk)), key=lambda i: sb[i] < 0)
            slot_k[t] = [sk[i] for i in osort]
            slot_dA[t] = [sa[i] for i in osort]
            slot_dB[t] = [sb[i] for i in osort]

        cores.append(dict(slot_k=slot_k, slot_dA=slot_dA, slot_dB=slot_dB,
                          table=table, n_units=n_units))

    # shared per-tile quotas and block map
    quota = np.zeros(ntiles, np.int64)
    for t in range(ntiles):
        quota[t] = max(max(len(cores[c]["slot_k"][t]) for c in range(NCORES)), 1)
    cum = np.concatenate([[0], np.cumsum(quota)])
    nslot = int(cum[-1])
    nslot_pad = _rup(nslot, BLK)
    nblk = nslot_pad // BLK
    bstart = (cum[:-1] // BLK).astype(np.int64)
    bend = np.minimum(-(-cum[1:] // BLK), nblk).astype(np.int64)
    bend = np.maximum(bend, bstart + 1)
    # T0(b): first tile covering block b (tiles cover [bstart, bend), which
    # has no gaps since bend[t] >= bstart[t+1])
    T0 = np.zeros(nblk, np.int64)
    cur = 0
    for b in range(nblk):
        while bend[cur] <= b:
            cur += 1
        T0[b] = cur

    # per-core padded streams + active lane flags
    activeA = np.zeros((ntiles, nblk), bool)
    activeB = np.zeros((ntiles, nblk), bool)
    for c in range(NCORES):
        d = cores[c]
        kidx = np.zeros(nslot_pad, np.int64)
        dA = np.full(nslot_pad, -1.0, np.float32)
        dB = np.full(nslot_pad, -1.0, np.float32)
        for t in range(ntiles):
            off = int(cum[t])
            sk, sa, sb = d["slot_k"][t], d["slot_dA"][t], d["slot_dB"][t]
            n = len(sk)
            kidx[off:off + n] = sk
            # dl shifted relative to the covering block's T0
            for i in range(n):
                b = (off + i) // BLK
                shift = 128 * int(T0[b])
                dA[off + i] = sa[i] - shift
                activeA[t, b] = True
                if sb[i] >= 0:
                    dB[off + i] = sb[i] - shift
                    activeB[t, b] = True
        # relation tail: idx stays 0 (garbage gather, dl=-1 kills it);
        # trailing [nslot, nslot_pad) marked -1 (skipped via num_idxs_reg)
        kidx[nslot:] = -1
        d["kidx"], d["dA"], d["dB"] = kidx, dA, dB
        del d["slot_k"], d["slot_dA"], d["slot_dB"]

    # force one active matmul per tile so every agg gets a start+stop
    for t in range(ntiles):
        if not activeA[t, bstart[t]:bend[t]].any() and \
           not activeB[t, bstart[t]:bend[t]].any():
            activeA[t, bstart[t]] = True

    return dict(cores=cores, ntiles=ntiles, D=D, n_dst=n_dst,
                nslot=nslot, nslot_pad=nslot_pad, nblk=nblk,
                bstart=bstart, bend=bend, T0=T0,
                activeA=activeA, activeB=activeB)


def _build_host_data(inputs):
    def prescale(feat, src, n_src):
        deg = np.maximum(np.bincount(np.asarray(src, np.int64),
                                     minlength=n_src), 1.0)
        return (np.asarray(feat, np.float32)
                / np.sqrt(deg)[:, None]).astype(np.float32)

    feat0 = prescale(inputs["instance_feat"], inputs["in_src"], INST_N)
    feat1 = prescale(inputs["node_feat"], inputs["ni_src"], NODE_N)
    feat2 = prescale(inputs["svc_feat"], inputs["sc_src"], SVC_N)

    rels = [
        # order matters: output rows are [node_out, inst_out, svc_out]
        _prep_relation(inputs["in_src"], inputs["in_dst"], INST_N, NODE_N,
                       feat0, compact=True),
        _prep_relation(inputs["ni_src"], inputs["ni_dst"], NODE_N, INST_N,
                       feat1, compact=False),
        _prep_relation(inputs["sc_src"], inputs["sc_dst"], SVC_N, SVC_N,
                       feat2, compact=False),
    ]
    Ws = [inputs["W_inst"], inputs["W_node"], inputs["W_svc"]]
    bs = [inputs["b_inst"], inputs["b_node"], inputs["b_svc"]]

    def rs_ud(dstv, n_dst):
        deg = np.maximum(np.bincount(np.asarray(dstv, np.int64),
                                     minlength=n_dst), 1.0)
        return (1.0 / np.sqrt(deg)).astype(np.float32), \
            np.sqrt(deg).astype(np.float32)
    rs_all = [rs_ud(inputs["in_dst"], NODE_N),
              rs_ud(inputs["ni_dst"], INST_N),
              rs_ud(inputs["sc_dst"], SVC_N)]

    umax = _rup(max(r["cores"][c]["n_units"]
                    for r in rels[:1] for c in range(NCORES)) + 1, 16)
    nblk_tot = sum(r["nblk"] for r in rels)
    nidx_tot = nblk_tot * BLK
    ntile_tot = sum(r["ntiles"] for r in rels)

    W_cat = np.concatenate([np.asarray(w, np.float32) for w in Ws], axis=1)
    b_row = np.concatenate([np.asarray(b, np.float32) for b in bs])[None, :]

    # straddle span: max tiles covered by one block
    kmax = 1
    for r in rels:
        span = np.zeros(r["nblk"], np.int64)
        for t in range(r["ntiles"]):
            for b in range(int(r["bstart"][t]), int(r["bend"][t])):
                span[b] = max(span[b], t - int(r["T0"][b]) + 1)
        kmax = max(kmax, int(span.max()))
    assert kmax <= 16, f"straddle span {kmax} breaks fp16-exact iota"
    iota_eqg = np.tile(np.arange(128, dtype=np.float16), (128, EQG))
    iota_ramp = np.tile(np.arange(kmax * 128, dtype=np.float16), (128, 1))

    in_maps = []
    for c in range(NCORES):
        kidx = np.concatenate([r["cores"][c]["kidx"] for r in rels])
        dA = np.concatenate([r["cores"][c]["dA"] for r in rels])
        dB = np.concatenate([r["cores"][c]["dB"] for r in rels])
        assert kidx.max() < 32768
        idx16 = np.ascontiguousarray(kidx.astype(np.int16).reshape(-1, 16).T)
        idx_sb = np.tile(idx16, (8, 1))                          # [128, nidx/16]
        dA_sb = np.ascontiguousarray(
            dA.reshape(nblk_tot, BLK).T).astype(np.float32)      # [128, nblk]
        dB_sb = np.ascontiguousarray(
            dB.reshape(nblk_tot, BLK).T).astype(np.float32)

        rs_sb = np.zeros((128, ntile_tot), np.float32)
        u_sb = np.zeros((1, ntile_tot * 128), np.float32)
        t0 = 0
        for r, (rs_d, u_d) in zip(rels, rs_all):
            lo = c * r["D"]
            val_rs = np.zeros(r["D"], np.float32)
            val_u = np.zeros(r["D"], np.float32)
            n = max(0, min(r["D"], r["n_dst"] - lo))
            if n > 0:
                val_rs[:n] = rs_d[lo:lo + n]
                val_u[:n] = u_d[lo:lo + n]
            rs_sb[:, t0:t0 + r["ntiles"]] = val_rs.reshape(r["ntiles"], 128).T
            u_sb[0, t0 * 128:(t0 + r["ntiles"]) * 128] = val_u
            t0 += r["ntiles"]

        # tables (fp16, +1 pad row for the window overrun at the last unit)
        def mk_tbl(tab, rows):
            out = np.zeros((rows, HID), np.float16)
            out[:len(tab)] = tab.astype(np.float16)
            return np.ascontiguousarray(out)

        in_maps.append({
            "tbl_in": mk_tbl(rels[0]["cores"][c]["table"], umax),
            "tbl_ni": mk_tbl(rels[1]["cores"][c]["table"], NODE_N + 1),
            "tbl_sc": mk_tbl(rels[2]["cores"][c]["table"], SVC_N + 1),
            "idx_sb": np.ascontiguousarray(idx_sb),
            "dA_sb": dA_sb,
            "dB_sb": dB_sb,
            "rs_sb": rs_sb,
            "u_sb": u_sb,
            "W_cat": np.ascontiguousarray(W_cat),
            "b_row": np.ascontiguousarray(b_row),
            "iota_eqg": np.ascontiguousarray(iota_eqg),
            "iota_ramp": np.ascontiguousarray(iota_ramp),
        })

    meta = dict(
        umax=umax, nblk_tot=nblk_tot, nidx_tot=nidx_tot, ntile_tot=ntile_tot,
        kmax=kmax,
        ntiles=[r["ntiles"] for r in rels],
        Ds=[r["D"] for r in rels],
        n_dsts=[r["n_dst"] for r in rels],
        nslots=[r["nslot"] for r in rels],
        nblks=[r["nblk"] for r in rels],
        bstarts=[r["bstart"].tolist() for r in rels],
        bends=[r["bend"].tolist() for r in rels],
        T0s=[r["T0"].tolist() for r in rels],
        activeA=[r["activeA"] for r in rels],
        activeB=[r["activeB"] for r in rels],
        tbl_rows=[umax, NODE_N + 1, SVC_N + 1],
    )
    return meta, in_maps


def _build_program(meta):
    import concourse.bacc as bacc
    import concourse.mybir as mybir
    import concourse.tile as tile

    f16 = mybir.dt.float16
    f32 = mybir.dt.float32
    AF = mybir.ActivationFunctionType
    act_fn = AF.Lrelu if ACT_MODE == "lrelu" else AF.Relu

    nblk_tot, nidx_tot = meta["nblk_tot"], meta["nidx_tot"]
    ntile_tot, kmax = meta["ntile_tot"], meta["kmax"]

    nc = bacc.Bacc("TRN2", target_bir_lowering=False, debug=False,
                   enable_asserts=False, num_devices=NCORES)

    tbl_d = [
        nc.dram_tensor("tbl_in", [meta["tbl_rows"][0], HID], f16,
                       kind="ExternalInput"),
        nc.dram_tensor("tbl_ni", [meta["tbl_rows"][1], HID], f16,
                       kind="ExternalInput"),
        nc.dram_tensor("tbl_sc", [meta["tbl_rows"][2], HID], f16,
                       kind="ExternalInput"),
    ]
    idx_d = nc.dram_tensor("idx_sb", [128, nidx_tot // 16], mybir.dt.int16,
                           kind="ExternalInput")
    dA_d = nc.dram_tensor("dA_sb", [128, nblk_tot], f32, kind="ExternalInput")
    dB_d = nc.dram_tensor("dB_sb", [128, nblk_tot], f32, kind="ExternalInput")
    rs_d = nc.dram_tensor("rs_sb", [128, ntile_tot], f32, kind="ExternalInput")
    u_d = nc.dram_tensor("u_sb", [1, ntile_tot * 128], f32, kind="ExternalInput")
    W_d = nc.dram_tensor("W_cat", [128, 3 * HID], f32, kind="ExternalInput")
    b_d = nc.dram_tensor("b_row", [1, 3 * HID], f32, kind="ExternalInput")
    ioe_d = nc.dram_tensor("iota_eqg", [128, EQG * 128], f16,
                           kind="ExternalInput")
    ior_d = nc.dram_tensor("iota_ramp", [128, kmax * 128], f16,
                           kind="ExternalInput")

    out_d = [
        nc.dram_tensor(nm, [128, meta["ntiles"][i] * 128], f16,
                       kind="ExternalOutput")
        for i, nm in enumerate(["out_node", "out_inst", "out_svc"])
    ]

    with tile.TileContext(nc) as tc:
        with (
            tc.tile_pool(name="const", bufs=1) as const,
            tc.tile_pool(name="g", bufs=4) as gpool,
            tc.tile_pool(name="st", bufs=10) as stpool,
            tc.tile_pool(name="evac", bufs=4) as evac,
            tc.tile_pool(name="osb", bufs=4) as opool,
            tc.tile_pool(name="psA", bufs=6, space="PSUM") as psA,
            tc.tile_pool(name="psO", bufs=2, space="PSUM") as psO,
        ):
            # load the leading idx slice first so gathers start ASAP
            idx_t = const.tile([128, nidx_tot // 16], mybir.dt.int16)
            c0 = min(2 * CHUNK * BLK // 16, nidx_tot // 16)
            nc.sync.dma_start(idx_t[:, :c0], idx_d.ap()[:, :c0])
            dA_t = const.tile([128, nblk_tot], f32)
            nc.sync.dma_start(dA_t[:], dA_d.ap())
            dB_t = const.tile([128, nblk_tot], f32)
            nc.sync.dma_start(dB_t[:], dB_d.ap())
            ioe_t = const.tile([128, EQG * 128], f16)
            nc.sync.dma_start(ioe_t[:], ioe_d.ap())
            ior_t = const.tile([128, kmax * 128], f16)
            nc.sync.dma_start(ior_t[:], ior_d.ap())
            W_t = const.tile([128, 3 * HID], f32)
            nc.sync.dma_start(W_t[:], W_d.ap())
            b_t = const.tile([1, 3 * HID], f32)
            nc.sync.dma_start(b_t[:], b_d.ap())
            u_t = const.tile([1, ntile_tot * 128], f32)
            nc.sync.dma_start(u_t[:], u_d.ap())
            rs_t = const.tile([128, ntile_tot], f32)
            nc.sync.dma_start(rs_t[:], rs_d.ap())
            if c0 < nidx_tot // 16:
                nc.sync.dma_start(idx_t[:, c0:], idx_d.ap()[:, c0:])

            g_tiles = {}    # global chunk -> gather tile
            st0_tiles = {}  # (group, lane) -> batched k=0 one-hot
            stk_tiles = {}  # (block, lane, k) -> straddle one-hot

            def issue_gather(ci, rel, rel_cblk0, rel_blk0, rel_nblk, rel_nslot):
                local_b0 = (ci - rel_cblk0) * CHUNK
                cblk = min(CHUNK, rel_nblk - local_b0)
                gt = gpool.tile([128, CHUNK, LANES * HID], f16, tag="g")
                nidx = cblk * BLK
                reg = min(rel_nslot - local_b0 * BLK, nidx)
                off16 = (rel_blk0 + local_b0) * BLK // 16
                in_ap = tbl_d[rel].ap()
                n_units = meta["tbl_rows"][rel] - 1
                in_ap.ap[0] = [HID, n_units]
                in_ap.ap[1] = [1, LANES * HID]
                nc.gpsimd.dma_gather(
                    out_ap=gt[:, :cblk, :],
                    in_ap=in_ap,
                    idxs_ap=idx_t[:, off16:off16 + nidx // 16],
                    num_idxs=nidx,
                    num_idxs_reg=reg,
                    elem_size=LANES * HID,
                    elem_step=HID,
                    single_packet=False,
                )
                g_tiles[ci] = gt

            def issue_st0(gi, lane, dl_t):
                g0 = gi * EQG
                gsz = min(EQG, nblk_tot - g0)
                st = stpool.tile([128, EQG, 128], f16, tag="st")
                nc.vector.tensor_tensor(
                    st[:, :gsz, :],
                    ioe_t[:, :gsz * 128].rearrange("p (g k) -> p g k", k=128),
                    dl_t[:, g0:g0 + gsz].broadcast_to([128, gsz, 128]),
                    mybir.AluOpType.is_equal)
                st0_tiles[(gi, lane)] = st

            def issue_stk(b, lane, k, dl_t):
                st = stpool.tile([128, 128], f16, tag="stk")
                nc.vector.tensor_scalar(
                    st[:], ior_t[:, k * 128:(k + 1) * 128],
                    dl_t[:, b:b + 1], None, mybir.AluOpType.is_equal)
                stk_tiles[(b, lane, k)] = st

            cblk_base = 0   # global chunk index base for this relation
            blk_base = 0    # global block index base
            tglob = 0
            for rel in range(3):
                ntiles = meta["ntiles"][rel]
                nblk = meta["nblks"][rel]
                nslot = meta["nslots"][rel]
                bstart = meta["bstarts"][rel]
                bend = meta["bends"][rel]
                T0 = meta["T0s"][rel]
                actA = meta["activeA"][rel]
                actB = meta["activeB"][rel]
                for t in range(ntiles):
                    # emission list: (block, lane) pairs for this tile
                    ems = []
                    for b in range(int(bstart[t]), int(bend[t])):
                        if actA[t, b]:
                            ems.append((b, 0))
                        if actB[t, b]:
                            ems.append((b, 1))
                    agg = psA.tile([128, 128], f32, tag="agg")
                    for i, (b, lane) in enumerate(ems):
                        gb = blk_base + b
                        ci = cblk_base + b // CHUNK
                        if ci not in g_tiles:
                            issue_gather(ci, rel, cblk_base, blk_base,
                                         nblk, nslot)
                        k = t - int(T0[b])
                        dl_t = dA_t if lane == 0 else dB_t
                        if k == 0:
                            gi = gb // EQG
                            if (gi, lane) not in st0_tiles:
                                issue_st0(gi, lane, dl_t)
                            st_ap = st0_tiles[(gi, lane)][:, gb % EQG, :]
                        else:
                            if (gb, lane, k) not in stk_tiles:
                                issue_stk(gb, lane, k, dl_t)
                            st_ap = stk_tiles[(gb, lane, k)][:]
                        cj = b % CHUNK
                        nc.tensor.matmul(
                            agg[:],
                            g_tiles[ci][:, cj, lane * HID:(lane + 1) * HID],
                            st_ap,
                            start=(i == 0), stop=(i == len(ems) - 1))
                    # epilogue
                    aggsb = evac.tile([128, 128], f32, tag="evac")
                    nc.scalar.copy(aggsb[:], agg[:])
                    po = psO.tile([128, 128], f32, tag="po")
                    nc.tensor.matmul(
                        po[:], u_t[:, tglob * 128:(tglob + 1) * 128],
                        b_t[:, rel * HID:(rel + 1) * HID],
                        start=True, stop=False, skip_group_check=True)
                    nc.tensor.matmul(
                        po[:], aggsb[:], W_t[:, rel * HID:(rel + 1) * HID],
                        start=False, stop=True, skip_group_check=True)
                    oj = t % OUT_GRP
                    if oj == 0:
                        osb = opool.tile([128, OUT_GRP, 128], f16, tag="osb")
                        osb_t0 = t
                    nc.scalar.activation(
                        osb[:, oj, :], po[:], act_fn,
                        bias=0.0, scale=rs_t[:, tglob:tglob + 1], alpha=0.01)
                    if oj == OUT_GRP - 1 or t == ntiles - 1:
                        cnt = t - osb_t0 + 1
                        dst = out_d[rel].ap()[:, osb_t0 * 128:(t + 1) * 128]
                        nc.sync.dma_start(
                            dst.rearrange("p (j k) -> p j k", k=128),
                            osb[:, :cnt, :])
                    tglob += 1
                cblk_base += _cdiv(nblk, CHUNK)
                blk_base += nblk

    nc.compile()
    return nc


def _run(nc, in_maps, trace=False, **kw):
    from concourse import bass_utils
    res = bass_utils.run_bass_kernel_spmd(
        nc, in_maps, core_ids=list(range(NCORES)), trace=trace, **kw)
    return res


def _assemble(results, meta):
    out = np.empty((NODE_N + INST_N + SVC_N, HID), np.float32)
    offs = [0, NODE_N, NODE_N + INST_N]
    names = ["out_node", "out_inst", "out_svc"]
    for rel in range(3):
        D, n_dst = meta["Ds"][rel], meta["n_dsts"][rel]
        ntiles = meta["ntiles"][rel]
        for c in range(NCORES):
            lo = c * D
            n = max(0, min(D, n_dst - lo))
            if n > 0:
                arr = results[c][names[rel]]  # [128, ntiles*128] fp16
                rows = np.ascontiguousarray(
                    arr.reshape(128, ntiles, 128).transpose(1, 0, 2)
                ).reshape(-1, HID)[:n].astype(np.float32)
                out[offs[rel] + lo: offs[rel] + lo + n] = rows
    return out


def kernel(**inputs):
    import hashlib
    key = "prog"
    h = hashlib.sha1()
    for k in ("sc_src", "sc_dst", "in_src", "in_dst", "ni_src", "ni_dst"):
        h.update(np.ascontiguousarray(np.asarray(inputs[k], np.int32)).tobytes())
    sig = h.hexdigest()
    meta, in_maps = _build_host_data(inputs)
    if key in _cache and _cache[key][0] == sig:
        _, nc, _ = _cache[key]
    else:
        nc = _build_program(meta)
        _cache[key] = (sig, nc, meta)
    res = _run(nc, in_maps)
    return _assemble(res.results, meta)


# revision 11
# speedup vs baseline: 1.1981x; 1.0050x over previous
"""Trainium2 Bass kernel for a heterogeneous GraphConv layer (3 relations).

out = concat([leaky(GC(inst_feat, W_inst, in_*)),     # -> node   (10000)
              leaky(GC(node_feat, W_node, ni_*)),     # -> inst   (100000)
              leaky(GC(svc_feat,  W_svc,  sc_*))])    # -> svc    (20000)

GC(f, W, src, dst) = rsqrt(deg_d) * segsum_dst((rsqrt(deg_s)*f)[src]) @ W + b
(aggregation commutes with the dense @W, so we gather *raw scaled features*
and apply W once per destination tile).

Strategy: destination-sharded across 8 NeuronCores.  The per-core source
tables are PERMUTED so that rows co-used by the same dst tile sit adjacently;
each dma_gather descriptor then uses an overlapping 512B window (elem 256
fp16 elems, step 128) that fetches TWO consecutive rows — one descriptor
serves up to two edges (lanes A/B).  Descriptor cost on TRN2 is identical
for 256B and 512B payloads, so pairing halves gather DMA time.

Edges (sorted by dst) are packed densely into 128-slot blocks with per-tile
slot quotas (max over cores) so the block->tile map is identical on every
core; blocks straddling a dst-tile boundary contribute to 2+ tiles via
shifted one-hots (iota + 128k).  Per block and lane: DVE builds a one-hot
S[slot,dst]=(dl==iota_k), PE accumulates agg[f,dst] += G_lane.T @ S in PSUM.
Per dst tile: PSUM out = u (x) b (rank-1 bias preload) + agg.T @ W, one
ScalarE Lrelu(out * rsqrt_deg_d), fp16 transposed output DMA (the host
de-transposes and converts).
"""

import os as _os
from collections import defaultdict

import numpy as np

SVC_N, INST_N, NODE_N, HID = 20000, 100000, 10000, 128
NCORES = 8
BLK = 128           # slots per block (= PE contraction dim)
LANES = 2           # table rows per gather window (512B / 256B fp16 rows)
CHUNK = int(_os.environ.get("GNN_CHUNK", "32"))   # blocks per gather instr
EQG = int(_os.environ.get("GNN_EQG", "4"))        # blocks per batched one-hot
OUT_GRP = int(_os.environ.get("GNN_OUT_GRP", "8"))  # dst tiles per out DMA
ACT_MODE = "lrelu"

_cache = {}


def _cdiv(a, b):
    return (a + b - 1) // b


def _rup(a, b):
    return _cdiv(a, b) * b


def _sequence_sources(es, tile):
    """Order this core's used sources so same-tileset sources are adjacent.

    es/tile: per-edge (sorted by (src, tile)).  Returns (seq_srcs, starts,
    ends) where seq_srcs is the source order and starts/ends delimit each
    source's edge range in the (src,tile)-sorted arrays.
    """
    n = len(es)
    starts = np.flatnonzero(np.r_[True, es[1:] != es[:-1]])
    ends = np.r_[starts[1:], n]
    keys = [tuple(tile[a:b]) for a, b in zip(starts, ends)]
    order = sorted(range(len(starts)), key=lambda i: keys[i])
    return order, starts, ends


def _prep_relation(src, dst, n_src, n_dst, feat_s, compact):
    """Host-side sharding/packing for one relation.

    Returns dict with per-core slot streams and shared structure info.
    """
    src = np.asarray(src, np.int64)
    dst = np.asarray(dst, np.int64)

    D = _rup(_cdiv(n_dst, NCORES), 128)  # dst rows per core (padded)
    ntiles = D // 128

    cores = []
    for c in range(NCORES):
        lo = c * D
        m = (dst >= lo) & (dst < lo + D)
        es, ed = src[m], dst[m] - lo
        tl = ed >> 7
        order = np.lexsort((tl, es))
        es, ed, tl = es[order], ed[order], tl[order]

        uorder, starts, ends = _sequence_sources(es, tl)
        srcs_u = es[starts]
        nsrc_u = len(srcs_u)

        # position of each unique source in the permuted table
        pos_of_u = np.empty(nsrc_u, np.int64)
        pos_of_u[uorder] = np.arange(nsrc_u)

        if compact:
            table = feat_s[srcs_u[uorder]]
            n_units = nsrc_u
        else:
            # full table permuted: used sources in sequence order first,
            # then unused rows
            used_mask = np.zeros(n_src, bool)
            used_mask[srcs_u] = True
            perm = np.concatenate([srcs_u[uorder],
                                   np.flatnonzero(~used_mask)])
            table = feat_s[perm]
            n_units = n_src

        # per tile: list of (position, dst_local) per edge
        # build slots tile by tile with the path-greedy pairing
        slot_k = [[] for _ in range(ntiles)]
        slot_dA = [[] for _ in range(ntiles)]
        slot_dB = [[] for _ in range(ntiles)]
        # gather per-(tile) positions: iterate unique srcs, distribute edges
        per_tile = defaultdict(list)  # tile -> list of (pos, [dst_locals])
        for ui in range(nsrc_u):
            a, b = starts[ui], ends[ui]
            p = pos_of_u[ui]
            # edges of this src grouped by tile (tl sorted within src range)
            t0 = a
            while t0 < b:
                t1 = t0
                while t1 < b and tl[t1] == tl[t0]:
                    t1 += 1
                per_tile[tl[t0]].append((p, ed[t0:t1]))
                t0 = t1
        for t, lst in per_tile.items():
            lst.sort(key=lambda x: x[0])
            sk, sa, sb = slot_k[t], slot_dA[t], slot_dB[t]
            prev_pos = -10
            prev_ds = []  # unpaired dst_locals of prev position
            for p, ds in lst:
                ds = list(ds)
                if p == prev_pos + 1 and prev_ds:
                    npair = min(len(prev_ds), len(ds))
                    for i in range(npair):
                        sk.append(prev_pos)
                        sa.append(prev_ds[i])
                        sb.append(ds[i])
                    for d in prev_ds[npair:]:
                        sk.append(prev_pos)
                        sa.append(d)
                        sb.append(-1)
                    ds = ds[npair:]
                else:
                    for d in prev_ds:
                        sk.append(prev_pos)
                        sa.append(d)
                        sb.append(-1)
                prev_pos, prev_ds = p, ds
            for d in prev_ds:
                sk.append(prev_pos)
                sa.append(d)
                sb.append(-1)
            # put paired slots first so lane-B tails can be skipped
            osort = sorted(range(len(sk)), key=lambda i: sb[i] < 0)
            slot_k[t] = [sk[i] for i in osort]
            slot_dA[t] = [sa[i] for i in osort]
            slot_dB[t] = [sb[i] for i in osort]

        cores.append(dict(slot_k=slot_k, slot_dA=slot_dA, slot_dB=slot_dB,
                          table=table, n_units=n_units))

    # shared per-tile quotas and block map
    quota = np.zeros(ntiles, np.int64)
    for t in range(ntiles):
        quota[t] = max(max(len(cores[c]["slot_k"][t]) for c in range(NCORES)), 1)
    cum = np.concatenate([[0], np.cumsum(quota)])
    nslot = int(cum[-1])
    nslot_pad = _rup(nslot, BLK)
    nblk = nslot_pad // BLK
    bstart = (cum[:-1] // BLK).astype(np.int64)
    bend = np.minimum(-(-cum[1:] // BLK), nblk).astype(np.int64)
    bend = np.maximum(bend, bstart + 1)
    # T0(b): first tile covering block b (tiles cover [bstart, bend), which
    # has no gaps since bend[t] >= bstart[t+1])
    T0 = np.zeros(nblk, np.int64)
    cur = 0
    for b in range(nblk):
        while bend[cur] <= b:
            cur += 1
        T0[b] = cur

    # per-core padded streams + active lane flags
    activeA = np.zeros((ntiles, nblk), bool)
    activeB = np.zeros((ntiles, nblk), bool)
    for c in range(NCORES):
        d = cores[c]
        kidx = np.zeros(nslot_pad, np.int64)
        dA = np.full(nslot_pad, -1.0, np.float32)
        dB = np.full(nslot_pad, -1.0, np.float32)
        for t in range(ntiles):
            off = int(cum[t])
            sk, sa, sb = d["slot_k"][t], d["slot_dA"][t], d["slot_dB"][t]
            n = len(sk)
            kidx[off:off + n] = sk
            # dl shifted relative to the covering block's T0
            for i in range(n):
                b = (off + i) // BLK
                shift = 128 * int(T0[b])
                dA[off + i] = sa[i] - shift
                activeA[t, b] = True
                if sb[i] >= 0:
                    dB[off + i] = sb[i] - shift
                    activeB[t, b] = True
        # relation tail: idx stays 0 (garbage gather, dl=-1 kills it);
        # trailing [nslot, nslot_pad) marked -1 (skipped via num_idxs_reg)
        kidx[nslot:] = -1
        d["kidx"], d["dA"], d["dB"] = kidx, dA, dB
        del d["slot_k"], d["slot_dA"], d["slot_dB"]

    # force one active matmul per tile so every agg gets a start+stop
    for t in range(ntiles):
        if not activeA[t, bstart[t]:bend[t]].any() and \
           not activeB[t, bstart[t]:bend[t]].any():
            activeA[t, bstart[t]] = True

    return dict(cores=cores, ntiles=ntiles, D=D, n_dst=n_dst,
                nslot=nslot, nslot_pad=nslot_pad, nblk=nblk,
                bstart=bstart, bend=bend, T0=T0,
                activeA=activeA, activeB=activeB)


def _build_host_data(inputs):
    def prescale(feat, src, n_src):
        deg = np.maximum(np.bincount(np.asarray(src, np.int64),
                                     minlength=n_src), 1.0)
        return (np.asarray(feat, np.float32)
                / np.sqrt(deg)[:, None]).astype(np.float32)

    feat0 = prescale(inputs["instance_feat"], inputs["in_src"], INST_N)
    feat1 = prescale(inputs["node_feat"], inputs["ni_src"], NODE_N)
    feat2 = prescale(inputs["svc_feat"], inputs["sc_src"], SVC_N)

    rels = [
        # order matters: output rows are [node_out, inst_out, svc_out]
        _prep_relation(inputs["in_src"], inputs["in_dst"], INST_N, NODE_N,
                       feat0, compact=True),
        _prep_relation(inputs["ni_src"], inputs["ni_dst"], NODE_N, INST_N,
                       feat1, compact=False),
        _prep_relation(inputs["sc_src"], inputs["sc_dst"], SVC_N, SVC_N,
                       feat2, compact=False),
    ]
    Ws = [inputs["W_inst"], inputs["W_node"], inputs["W_svc"]]
    bs = [inputs["b_inst"], inputs["b_node"], inputs["b_svc"]]

    def rs_ud(dstv, n_dst):
        deg = np.maximum(np.bincount(np.asarray(dstv, np.int64),
                                     minlength=n_dst), 1.0)
        return (1.0 / np.sqrt(deg)).astype(np.float32), \
            np.sqrt(deg).astype(np.float32)
    rs_all = [rs_ud(inputs["in_dst"], NODE_N),
              rs_ud(inputs["ni_dst"], INST_N),
              rs_ud(inputs["sc_dst"], SVC_N)]

    umax = _rup(max(r["cores"][c]["n_units"]
                    for r in rels[:1] for c in range(NCORES)) + 1, 16)
    nblk_tot = sum(r["nblk"] for r in rels)
    nidx_tot = nblk_tot * BLK
    ntile_tot = sum(r["ntiles"] for r in rels)

    W_cat = np.concatenate([np.asarray(w, np.float32) for w in Ws], axis=1)
    b_row = np.concatenate([np.asarray(b, np.float32) for b in bs])[None, :]

    # straddle span: max tiles covered by one block
    kmax = 1
    for r in rels:
        span = np.zeros(r["nblk"], np.int64)
        for t in range(r["ntiles"]):
            for b in range(int(r["bstart"][t]), int(r["bend"][t])):
                span[b] = max(span[b], t - int(r["T0"][b]) + 1)
        kmax = max(kmax, int(span.max()))
    assert kmax <= 16, f"straddle span {kmax} breaks fp16-exact iota"
    iota_ramp = np.tile(np.arange(kmax * 128, dtype=np.float16), (128, 1))

    in_maps = []
    for c in range(NCORES):
        kidx = np.concatenate([r["cores"][c]["kidx"] for r in rels])
        dA = np.concatenate([r["cores"][c]["dA"] for r in rels])
        dB = np.concatenate([r["cores"][c]["dB"] for r in rels])
        assert kidx.max() < 32768
        idx16 = np.ascontiguousarray(kidx.astype(np.int16).reshape(-1, 16).T)
        idx_sb = np.tile(idx16, (8, 1))                          # [128, nidx/16]
        dA_sb = np.ascontiguousarray(
            dA.reshape(nblk_tot, BLK).T).astype(np.float32)      # [128, nblk]
        dB_sb = np.ascontiguousarray(
            dB.reshape(nblk_tot, BLK).T).astype(np.float32)

        rs_sb = np.zeros((128, ntile_tot), np.float32)
        u_sb = np.zeros((1, ntile_tot * 128), np.float32)
        t0 = 0
        for r, (rs_d, u_d) in zip(rels, rs_all):
            lo = c * r["D"]
            val_rs = np.zeros(r["D"], np.float32)
            val_u = np.zeros(r["D"], np.float32)
            n = max(0, min(r["D"], r["n_dst"] - lo))
            if n > 0:
                val_rs[:n] = rs_d[lo:lo + n]
                val_u[:n] = u_d[lo:lo + n]
            rs_sb[:, t0:t0 + r["ntiles"]] = val_rs.reshape(r["ntiles"], 128).T
            u_sb[0, t0 * 128:(t0 + r["ntiles"]) * 128] = val_u
            t0 += r["ntiles"]

        # tables (fp16, +1 pad row for the window overrun at the last unit)
        def mk_tbl(tab, rows):
            out = np.zeros((rows, HID), np.float16)
            out[:len(tab)] = tab.astype(np.float16)
            return np.ascontiguousarray(out)

        in_maps.append({
            "tbl_in": mk_tbl(rels[0]["cores"][c]["table"], umax),
            "tbl_ni": mk_tbl(rels[1]["cores"][c]["table"], NODE_N + 1),
            "tbl_sc": mk_tbl(rels[2]["cores"][c]["table"], SVC_N + 1),
            "idx_sb": np.ascontiguousarray(idx_sb),
            "dA_sb": dA_sb,
            "dB_sb": dB_sb,
            "rs_sb": rs_sb,
            "u_sb": u_sb,
            "W_cat": np.ascontiguousarray(W_cat),
            "b_row": np.ascontiguousarray(b_row),
            "iota_ramp": np.ascontiguousarray(iota_ramp),
        })

    meta = dict(
        umax=umax, nblk_tot=nblk_tot, nidx_tot=nidx_tot, ntile_tot=ntile_tot,
        kmax=kmax,
        ntiles=[r["ntiles"] for r in rels],
        Ds=[r["D"] for r in rels],
        n_dsts=[r["n_dst"] for r in rels],
        nslots=[r["nslot"] for r in rels],
        nblks=[r["nblk"] for r in rels],
        bstarts=[r["bstart"].tolist() for r in rels],
        bends=[r["bend"].tolist() for r in rels],
        T0s=[r["T0"].tolist() for r in rels],
        activeA=[r["activeA"] for r in rels],
        activeB=[r["activeB"] for r in rels],
        tbl_rows=[umax, NODE_N + 1, SVC_N + 1],
    )
    return meta, in_maps


def _build_program(meta):
    import concourse.bacc as bacc
    import concourse.mybir as mybir
    import concourse.tile as tile

    f16 = mybir.dt.float16
    f32 = mybir.dt.float32
    AF = mybir.ActivationFunctionType
    act_fn = AF.Lrelu if ACT_MODE == "lrelu" else AF.Relu

    nblk_tot, nidx_tot = meta["nblk_tot"], meta["nidx_tot"]
    ntile_tot, kmax = meta["ntile_tot"], meta["kmax"]

    nc = bacc.Bacc("TRN2", target_bir_lowering=False, debug=False,
                   enable_asserts=False, num_devices=NCORES)

    tbl_d = [
        nc.dram_tensor("tbl_in", [meta["tbl_rows"][0], HID], f16,
                       kind="ExternalInput"),
        nc.dram_tensor("tbl_ni", [meta["tbl_rows"][1], HID], f16,
                       kind="ExternalInput"),
        nc.dram_tensor("tbl_sc", [meta["tbl_rows"][2], HID], f16,
                       kind="ExternalInput"),
    ]
    idx_d = nc.dram_tensor("idx_sb", [128, nidx_tot // 16], mybir.dt.int16,
                           kind="ExternalInput")
    dA_d = nc.dram_tensor("dA_sb", [128, nblk_tot], f32, kind="ExternalInput")
    dB_d = nc.dram_tensor("dB_sb", [128, nblk_tot], f32, kind="ExternalInput")
    rs_d = nc.dram_tensor("rs_sb", [128, ntile_tot], f32, kind="ExternalInput")
    u_d = nc.dram_tensor("u_sb", [1, ntile_tot * 128], f32, kind="ExternalInput")
    W_d = nc.dram_tensor("W_cat", [128, 3 * HID], f32, kind="ExternalInput")
    b_d = nc.dram_tensor("b_row", [1, 3 * HID], f32, kind="ExternalInput")
    ior_d = nc.dram_tensor("iota_ramp", [128, kmax * 128], f16,
                           kind="ExternalInput")

    out_d = [
        nc.dram_tensor(nm, [128, meta["ntiles"][i] * 128], f16,
                       kind="ExternalOutput")
        for i, nm in enumerate(["out_node", "out_inst", "out_svc"])
    ]

    with tile.TileContext(nc) as tc:
        with (
            tc.tile_pool(name="const", bufs=1) as const,
            tc.tile_pool(name="g", bufs=4) as gpool,
            tc.tile_pool(name="st", bufs=10) as stpool,
            tc.tile_pool(name="evac", bufs=4) as evac,
            tc.tile_pool(name="osb", bufs=4) as opool,
            tc.tile_pool(name="psA", bufs=6, space="PSUM") as psA,
            tc.tile_pool(name="psO", bufs=2, space="PSUM") as psO,
        ):
            # load the leading idx slice first so gathers start ASAP
            idx_t = const.tile([128, nidx_tot // 16], mybir.dt.int16)
            c0 = min(2 * CHUNK * BLK // 16, nidx_tot // 16)
            nc.sync.dma_start(idx_t[:, :c0], idx_d.ap()[:, :c0])
            dA_t = const.tile([128, nblk_tot], f32)
            nc.sync.dma_start(dA_t[:], dA_d.ap())
            dB_t = const.tile([128, nblk_tot], f32)
            nc.sync.dma_start(dB_t[:], dB_d.ap())
            ior_t = const.tile([128, kmax * 128], f16)
            nc.sync.dma_start(ior_t[:], ior_d.ap())
            W_t = const.tile([128, 3 * HID], f32)
            nc.sync.dma_start(W_t[:], W_d.ap())
            b_t = const.tile([1, 3 * HID], f32)
            nc.sync.dma_start(b_t[:], b_d.ap())
            u_t = const.tile([1, ntile_tot * 128], f32)
            nc.sync.dma_start(u_t[:], u_d.ap())
            rs_t = const.tile([128, ntile_tot], f32)
            nc.sync.dma_start(rs_t[:], rs_d.ap())
            if c0 < nidx_tot // 16:
                nc.sync.dma_start(idx_t[:, c0:], idx_d.ap()[:, c0:])

            g_tiles = {}    # global chunk -> gather tile
            stk_tiles = {}  # (block, lane, k) -> one-hot [128,128]

            def issue_gather(ci, rel, rel_cblk0, rel_blk0, rel_nblk, rel_nslot):
                local_b0 = (ci - rel_cblk0) * CHUNK
                cblk = min(CHUNK, rel_nblk - local_b0)
                gt = gpool.tile([128, CHUNK, LANES * HID], f16, tag="g")
                nidx = cblk * BLK
                reg = min(rel_nslot - local_b0 * BLK, nidx)
                off16 = (rel_blk0 + local_b0) * BLK // 16
                in_ap = tbl_d[rel].ap()
                n_units = meta["tbl_rows"][rel] - 1
                in_ap.ap[0] = [HID, n_units]
                in_ap.ap[1] = [1, LANES * HID]
                nc.gpsimd.dma_gather(
                    out_ap=gt[:, :cblk, :],
                    in_ap=in_ap,
                    idxs_ap=idx_t[:, off16:off16 + nidx // 16],
                    num_idxs=nidx,
                    num_idxs_reg=reg,
                    elem_size=LANES * HID,
                    elem_step=HID,
                    single_packet=False,
                )
                g_tiles[ci] = gt

            def issue_stk(b, lane, k, dl_t):
                # TensorScalarPtr hits the 4x_2p DVE mode (fp16 packed in/out,
                # f32 per-partition scalar is exempt) — ~4x faster per element
                # than a broadcast TensorTensor, which only rates 1x.
                st = stpool.tile([128, 128], f16, tag="stk")
                nc.vector.tensor_scalar(
                    st[:], ior_t[:, k * 128:(k + 1) * 128],
                    dl_t[:, b:b + 1], None, mybir.AluOpType.is_equal)
                stk_tiles[(b, lane, k)] = st

            cblk_base = 0   # global chunk index base for this relation
            blk_base = 0    # global block index base
            tglob = 0
            for rel in range(3):
                ntiles = meta["ntiles"][rel]
                nblk = meta["nblks"][rel]
                nslot = meta["nslots"][rel]
                bstart = meta["bstarts"][rel]
                bend = meta["bends"][rel]
                T0 = meta["T0s"][rel]
                actA = meta["activeA"][rel]
                actB = meta["activeB"][rel]
                for t in range(ntiles):
                    # emission list: (block, lane) pairs for this tile
                    ems = []
                    for b in range(int(bstart[t]), int(bend[t])):
                        if actA[t, b]:
                            ems.append((b, 0))
                        if actB[t, b]:
                            ems.append((b, 1))
                    agg = psA.tile([128, 128], f32, tag="agg")
                    for i, (b, lane) in enumerate(ems):
                        gb = blk_base + b
                        ci = cblk_base + b // CHUNK
                        if ci not in g_tiles:
                            issue_gather(ci, rel, cblk_base, blk_base,
                                         nblk, nslot)
                        k = t - int(T0[b])
                        dl_t = dA_t if lane == 0 else dB_t
                        if (gb, lane, k) not in stk_tiles:
                            issue_stk(gb, lane, k, dl_t)
                        st_ap = stk_tiles[(gb, lane, k)][:]
                        cj = b % CHUNK
                        nc.tensor.matmul(
                            agg[:],
                            g_tiles[ci][:, cj, lane * HID:(lane + 1) * HID],
                            st_ap,
                            start=(i == 0), stop=(i == len(ems) - 1))
                    # epilogue
                    aggsb = evac.tile([128, 128], f32, tag="evac")
                    nc.scalar.copy(aggsb[:], agg[:])
                    po = psO.tile([128, 128], f32, tag="po")
                    nc.tensor.matmul(
                        po[:], u_t[:, tglob * 128:(tglob + 1) * 128],
                        b_t[:, rel * HID:(rel + 1) * HID],
                        start=True, stop=False, skip_group_check=True)
                    nc.tensor.matmul(
                        po[:], aggsb[:], W_t[:, rel * HID:(rel + 1) * HID],
                        start=False, stop=True, skip_group_check=True)
                    oj = t % OUT_GRP
                    if oj == 0:
                        osb = opool.tile([128, OUT_GRP, 128], f16, tag="osb")
                        osb_t0 = t
                    nc.scalar.activation(
                        osb[:, oj, :], po[:], act_fn,
                        bias=0.0, scale=rs_t[:, tglob:tglob + 1], alpha=0.01)
                    if oj == OUT_GRP - 1 or t == ntiles - 1:
                        cnt = t - osb_t0 + 1
                        dst = out_d[rel].ap()[:, osb_t0 * 128:(t + 1) * 128]
                        nc.sync.dma_start(
                            dst.rearrange("p (j k) -> p j k", k=128),
                            osb[:, :cnt, :])
                    tglob += 1
                cblk_base += _cdiv(nblk, CHUNK)
                blk_base += nblk

    nc.compile()
    return nc


def _run(nc, in_maps, trace=False, **kw):
    from concourse import bass_utils
    res = bass_utils.run_bass_kernel_spmd(
        nc, in_maps, core_ids=list(range(NCORES)), trace=trace, **kw)
    return res


def _assemble(results, meta):
    out = np.empty((NODE_N + INST_N + SVC_N, HID), np.float32)
    offs = [0, NODE_N, NODE_N + INST_N]
    names = ["out_node", "out_inst", "out_svc"]
    for rel in range(3):
        D, n_dst = meta["Ds"][rel], meta["n_dsts"][rel]
        ntiles = meta["ntiles"][rel]
        for c in range(NCORES):
            lo = c * D
            n = max(0, min(D, n_dst - lo))
            if n > 0:
                arr = results[c][names[rel]]  # [128, ntiles*128] fp16
                rows = np.ascontiguousarray(
                    arr.reshape(128, ntiles, 128).transpose(1, 0, 2)
                ).reshape(-1, HID)[:n].astype(np.float32)
                out[offs[rel] + lo: offs[rel] + lo + n] = rows
    return out


def kernel(**inputs):
    import hashlib
    key = "prog"
    h = hashlib.sha1()
    for k in ("sc_src", "sc_dst", "in_src", "in_dst", "ni_src", "ni_dst"):
        h.update(np.ascontiguousarray(np.asarray(inputs[k], np.int32)).tobytes())
    sig = h.hexdigest()
    meta, in_maps = _build_host_data(inputs)
    if key in _cache and _cache[key][0] == sig:
        _, nc, _ = _cache[key]
    else:
        nc = _build_program(meta)
        _cache[key] = (sig, nc, meta)
    res = _run(nc, in_maps)
    return _assemble(res.results, meta)


# revision 15
# speedup vs baseline: 1.2648x; 1.0556x over previous
"""Trainium2 Bass kernel for a heterogeneous GraphConv layer (3 relations).

out = concat([leaky(GC(inst_feat, W_inst, in_*)),     # -> node   (10000)
              leaky(GC(node_feat, W_node, ni_*)),     # -> inst   (100000)
              leaky(GC(svc_feat,  W_svc,  sc_*))])    # -> svc    (20000)

GC(f, W, src, dst) = rsqrt(deg_d) * segsum_dst((rsqrt(deg_s)*f)[src]) @ W + b
(aggregation commutes with the dense @W, so we gather *raw scaled features*
and apply W once per destination tile group).

Strategy: destination-sharded across 8 NeuronCores.  The per-core source
tables are PERMUTED so that rows co-used by the same dst tile sit adjacently;
each dma_gather descriptor then uses an overlapping 512B window (elem 256
fp16 elems, step 128) that fetches TWO consecutive rows — one descriptor
serves up to two edges (lanes A/B).  Descriptor cost on TRN2 is identical
for 256B and 512B payloads, so pairing halves gather DMA time.  Gathers use
prepare_only + trigger_dma so descriptor generation (Pool) pipelines with
the DMA transfers instead of holding Pool.SEQ through them.

Edges (sorted by dst) are packed densely into 128-slot blocks with per-tile
slot quotas (max over cores) so the block->tile map is identical on every
core.  Aggregation runs per GROUP of TP=2 dst tiles (256 PSUM columns):
per (block, lane, group) one DVE tensor_scalar builds a value-weighted
one-hot S[slot, d] = rs_dst * (dl == iota+off) (4x_2p DVE mode; the rsqrt
deg_d scale rides the one-hot so the epilogue needs no rank-1 bias matmul),
and PE accumulates agg[f, d] += G_lane.T @ S in PSUM.  Per group: one
matmul po[h, d] = W.T @ agg, one ScalarE Lrelu(po + b[h]) (bias per
partition in the [h, d] orientation), fp16 output DMA in the transposed
[h, d] layout (the host de-transposes and converts).
"""

import os as _os
from collections import defaultdict

import numpy as np

SVC_N, INST_N, NODE_N, HID = 20000, 100000, 10000, 128
NCORES = 8
BLK = 128           # slots per block (= PE contraction dim)
LANES = 2           # table rows per gather window (512B / 256B fp16 rows)
TP = 2              # dst tiles per aggregation group (256 PSUM columns)
CHUNK = int(_os.environ.get("GNN_CHUNK", "32"))   # blocks per gather instr
OUT_GRP = int(_os.environ.get("GNN_OUT_GRP", "8"))  # dst tiles per out DMA
ACT_MODE = "lrelu"

_cache = {}


def _cdiv(a, b):
    return (a + b - 1) // b


def _rup(a, b):
    return _cdiv(a, b) * b


def _sequence_sources(es, tile):
    """Order this core's used sources so same-tileset sources are adjacent."""
    n = len(es)
    starts = np.flatnonzero(np.r_[True, es[1:] != es[:-1]])
    ends = np.r_[starts[1:], n]
    keys = [tuple(tile[a:b]) for a, b in zip(starts, ends)]
    order = sorted(range(len(starts)), key=lambda i: keys[i])
    return order, starts, ends


def _prep_relation(src, dst, n_src, n_dst, feat_s, rs_d, compact):
    """Host-side sharding/packing for one relation."""
    src = np.asarray(src, np.int64)
    dst = np.asarray(dst, np.int64)

    D = _rup(_cdiv(n_dst, NCORES), 128)  # dst rows per core (padded)
    ntiles = D // 128
    assert ntiles % TP == 0

    cores = []
    for c in range(NCORES):
        lo = c * D
        m = (dst >= lo) & (dst < lo + D)
        es, ed = src[m], dst[m] - lo
        tl = ed >> 7
        order = np.lexsort((tl, es))
        es, ed, tl = es[order], ed[order], tl[order]

        uorder, starts, ends = _sequence_sources(es, tl)
        srcs_u = es[starts]
        nsrc_u = len(srcs_u)

        pos_of_u = np.empty(nsrc_u, np.int64)
        pos_of_u[uorder] = np.arange(nsrc_u)

        if compact:
            table = feat_s[srcs_u[uorder]]
            n_units = nsrc_u
        else:
            used_mask = np.zeros(n_src, bool)
            used_mask[srcs_u] = True
            perm = np.concatenate([srcs_u[uorder],
                                   np.flatnonzero(~used_mask)])
            table = feat_s[perm]
            n_units = n_src

        # slots per tile via the path-greedy pairing over table positions
        slot_k = [[] for _ in range(ntiles)]
        slot_dA = [[] for _ in range(ntiles)]
        slot_dB = [[] for _ in range(ntiles)]
        per_tile = defaultdict(list)  # tile -> list of (pos, [dst_locals])
        for ui in range(nsrc_u):
            a, b = starts[ui], ends[ui]
            p = pos_of_u[ui]
            t0 = a
            while t0 < b:
                t1 = t0
                while t1 < b and tl[t1] == tl[t0]:
                    t1 += 1
                per_tile[tl[t0]].append((p, ed[t0:t1]))
                t0 = t1
        for t, lst in per_tile.items():
            lst.sort(key=lambda x: x[0])
            sk, sa, sb = slot_k[t], slot_dA[t], slot_dB[t]
            prev_pos = -10
            prev_ds = []
            for p, ds in lst:
                ds = list(ds)
                if p == prev_pos + 1 and prev_ds:
                    npair = min(len(prev_ds), len(ds))
                    for i in range(npair):
                        sk.append(prev_pos)
                        sa.append(prev_ds[i])
                        sb.append(ds[i])
                    for d in prev_ds[npair:]:
                        sk.append(prev_pos)
                        sa.append(d)
                        sb.append(-1)
                    ds = ds[npair:]
                else:
                    for d in prev_ds:
                        sk.append(prev_pos)
                        sa.append(d)
                        sb.append(-1)
                prev_pos, prev_ds = p, ds
            for d in prev_ds:
                sk.append(prev_pos)
                sa.append(d)
                sb.append(-1)
            # paired slots first so lane-B tails can be skipped
            osort = sorted(range(len(sk)), key=lambda i: sb[i] < 0)
            slot_k[t] = [sk[i] for i in osort]
            slot_dA[t] = [sa[i] for i in osort]
            slot_dB[t] = [sb[i] for i in osort]

        cores.append(dict(slot_k=slot_k, slot_dA=slot_dA, slot_dB=slot_dB,
                          table=table, n_units=n_units))

    # shared per-tile quotas and block map
    quota = np.zeros(ntiles, np.int64)
    for t in range(ntiles):
        quota[t] = max(max(len(cores[c]["slot_k"][t]) for c in range(NCORES)), 1)
    cum = np.concatenate([[0], np.cumsum(quota)])
    nslot = int(cum[-1])
    nslot_pad = _rup(nslot, BLK)
    nblk = nslot_pad // BLK
    bstart = (cum[:-1] // BLK).astype(np.int64)
    bend = np.minimum(-(-cum[1:] // BLK), nblk).astype(np.int64)
    bend = np.maximum(bend, bstart + 1)
    # T0(b): first tile covering block b; G0(b): its group
    T0 = np.zeros(nblk, np.int64)
    cur = 0
    for b in range(nblk):
        while bend[cur] <= b:
            cur += 1
        T0[b] = cur
    G0 = T0 // TP

    # per-core dst rsqrt-degree values (0 beyond n_dst)
    rs_core = []
    for c in range(NCORES):
        lo = c * D
        v = np.zeros(D, np.float32)
        n = max(0, min(D, n_dst - lo))
        if n > 0:
            v[:n] = rs_d[lo:lo + n]
        rs_core.append(v)

    ngrp = ntiles // TP
    activeA = np.zeros((ngrp, nblk), bool)
    activeB = np.zeros((ngrp, nblk), bool)
    for c in range(NCORES):
        d = cores[c]
        kidx = np.zeros(nslot_pad, np.int64)
        dA = np.full(nslot_pad, -1.0, np.float32)
        dB = np.full(nslot_pad, -1.0, np.float32)
        rA = np.zeros(nslot_pad, np.float32)
        rB = np.zeros(nslot_pad, np.float32)
        rsv = rs_core[c]
        for t in range(ntiles):
            off = int(cum[t])
            sk, sa, sb = d["slot_k"][t], d["slot_dA"][t], d["slot_dB"][t]
            g = t // TP
            for i in range(len(sk)):
                b = (off + i) // BLK
                shift = 128 * TP * int(G0[b])
                kidx[off + i] = sk[i]
                dA[off + i] = sa[i] - shift
                rA[off + i] = rsv[sa[i]]
                activeA[g, b] = True
                if sb[i] >= 0:
                    dB[off + i] = sb[i] - shift
                    rB[off + i] = rsv[sb[i]]
                    activeB[g, b] = True
        # tail pads keep idx 0 (cost model charges num_idxs regardless; a
        # real gather keeps the SBUF block initialized -- NaN x 0 hazard)
        d["kidx"], d["dA"], d["dB"], d["rA"], d["rB"] = kidx, dA, dB, rA, rB
        del d["slot_k"], d["slot_dA"], d["slot_dB"]

    # force one active matmul per group so every agg gets a start+stop
    for g in range(ngrp):
        gbs = int(bstart[g * TP])
        gbe = int(bend[(g + 1) * TP - 1])
        if not activeA[g, gbs:gbe].any() and not activeB[g, gbs:gbe].any():
            activeA[g, gbs] = True

    return dict(cores=cores, ntiles=ntiles, ngrp=ngrp, D=D, n_dst=n_dst,
                nslot=nslot, nslot_pad=nslot_pad, nblk=nblk,
                bstart=bstart, bend=bend, G0=G0,
                activeA=activeA, activeB=activeB)


def _build_host_data(inputs):
    def prescale(feat, src, n_src):
        deg = np.maximum(np.bincount(np.asarray(src, np.int64),
                                     minlength=n_src), 1.0)
        return (np.asarray(feat, np.float32)
                / np.sqrt(deg)[:, None]).astype(np.float32)

    def rs_of(dstv, n_dst):
        deg = np.maximum(np.bincount(np.asarray(dstv, np.int64),
                                     minlength=n_dst), 1.0)
        return (1.0 / np.sqrt(deg)).astype(np.float32)

    feat0 = prescale(inputs["instance_feat"], inputs["in_src"], INST_N)
    feat1 = prescale(inputs["node_feat"], inputs["ni_src"], NODE_N)
    feat2 = prescale(inputs["svc_feat"], inputs["sc_src"], SVC_N)

    rels = [
        # order matters: output rows are [node_out, inst_out, svc_out]
        _prep_relation(inputs["in_src"], inputs["in_dst"], INST_N, NODE_N,
                       feat0, rs_of(inputs["in_dst"], NODE_N), compact=True),
        _prep_relation(inputs["ni_src"], inputs["ni_dst"], NODE_N, INST_N,
                       feat1, rs_of(inputs["ni_dst"], INST_N), compact=False),
        _prep_relation(inputs["sc_src"], inputs["sc_dst"], SVC_N, SVC_N,
                       feat2, rs_of(inputs["sc_dst"], SVC_N), compact=False),
    ]
    Ws = [inputs["W_inst"], inputs["W_node"], inputs["W_svc"]]
    bs = [inputs["b_inst"], inputs["b_node"], inputs["b_svc"]]

    umax = _rup(max(c["n_units"] for c in rels[0]["cores"]) + 2, 16)
    nblk_tot = sum(r["nblk"] for r in rels)
    nidx_tot = nblk_tot * BLK

    W_cat = np.concatenate([np.asarray(w, np.float32) for w in Ws], axis=1)
    b_col = np.stack([np.asarray(b, np.float32) for b in bs], axis=1)  # [128,3]

    # ramp width: max group span of any block, in groups
    kgmax = 1
    for r in rels:
        for g in range(r["ngrp"]):
            gbs, gbe = int(r["bstart"][g * TP]), int(r["bend"][(g + 1) * TP - 1])
            for b in range(gbs, gbe):
                kgmax = max(kgmax, g - int(r["G0"][b]) + 1)
    assert kgmax * TP * 128 <= 2048, f"ramp {kgmax * TP * 128} not fp16-exact"
    iota_ramp = np.tile(np.arange(kgmax * TP * 128, dtype=np.float16), (128, 1))

    in_maps = []
    for c in range(NCORES):
        kidx = np.concatenate([r["cores"][c]["kidx"] for r in rels])
        assert kidx.max() < 32768
        idx16 = np.ascontiguousarray(kidx.astype(np.int16).reshape(-1, 16).T)
        idx_sb = np.tile(idx16, (8, 1))

        def blkmaj(name):
            v = np.concatenate([r["cores"][c][name] for r in rels])
            return np.ascontiguousarray(
                v.reshape(nblk_tot, BLK).T).astype(np.float32)

        def mk_tbl(tab, rows):
            out = np.zeros((rows, HID), np.float16)
            out[:len(tab)] = tab.astype(np.float16)
            return np.ascontiguousarray(out)

        in_maps.append({
            "tbl_in": mk_tbl(rels[0]["cores"][c]["table"], umax),
            "tbl_ni": mk_tbl(rels[1]["cores"][c]["table"], NODE_N + 2),
            "tbl_sc": mk_tbl(rels[2]["cores"][c]["table"], SVC_N + 2),
            "idx_sb": np.ascontiguousarray(idx_sb),
            "dA_sb": blkmaj("dA"),
            "dB_sb": blkmaj("dB"),
            "rA_sb": blkmaj("rA"),
            "rB_sb": blkmaj("rB"),
            "W_cat": np.ascontiguousarray(W_cat),
            "b_col": np.ascontiguousarray(b_col),
            "iota_ramp": np.ascontiguousarray(iota_ramp),
        })

    meta = dict(
        umax=umax, nblk_tot=nblk_tot, nidx_tot=nidx_tot, kgmax=kgmax,
        ntiles=[r["ntiles"] for r in rels],
        ngrps=[r["ngrp"] for r in rels],
        Ds=[r["D"] for r in rels],
        n_dsts=[r["n_dst"] for r in rels],
        nslots=[r["nslot"] for r in rels],
        nblks=[r["nblk"] for r in rels],
        bstarts=[r["bstart"].tolist() for r in rels],
        bends=[r["bend"].tolist() for r in rels],
        G0s=[r["G0"].tolist() for r in rels],
        activeA=[r["activeA"] for r in rels],
        activeB=[r["activeB"] for r in rels],
        tbl_rows=[umax, NODE_N + 2, SVC_N + 2],
    )
    return meta, in_maps


def _build_program(meta):
    import concourse.bacc as bacc
    import concourse.mybir as mybir
    import concourse.tile as tile

    f16 = mybir.dt.float16
    f32 = mybir.dt.float32
    AF = mybir.ActivationFunctionType
    act_fn = AF.Lrelu if ACT_MODE == "lrelu" else AF.Relu

    nblk_tot, nidx_tot = meta["nblk_tot"], meta["nidx_tot"]
    kgmax = meta["kgmax"]
    GW = TP * 128  # group width in dst columns

    nc = bacc.Bacc("TRN2", target_bir_lowering=False, debug=False,
                   enable_asserts=False, num_devices=NCORES)

    tbl_d = [
        nc.dram_tensor(nm, [meta["tbl_rows"][i], HID], f16,
                       kind="ExternalInput")
        for i, nm in enumerate(["tbl_in", "tbl_ni", "tbl_sc"])
    ]
    idx_d = nc.dram_tensor("idx_sb", [128, nidx_tot // 16], mybir.dt.int16,
                           kind="ExternalInput")
    dA_d = nc.dram_tensor("dA_sb", [128, nblk_tot], f32, kind="ExternalInput")
    dB_d = nc.dram_tensor("dB_sb", [128, nblk_tot], f32, kind="ExternalInput")
    rA_d = nc.dram_tensor("rA_sb", [128, nblk_tot], f32, kind="ExternalInput")
    rB_d = nc.dram_tensor("rB_sb", [128, nblk_tot], f32, kind="ExternalInput")
    W_d = nc.dram_tensor("W_cat", [128, 3 * HID], f32, kind="ExternalInput")
    b_d = nc.dram_tensor("b_col", [128, 3], f32, kind="ExternalInput")
    ior_d = nc.dram_tensor("iota_ramp", [128, kgmax * GW], f16,
                           kind="ExternalInput")

    out_d = [
        nc.dram_tensor(nm, [128, meta["ntiles"][i] * 128], f16,
                       kind="ExternalOutput")
        for i, nm in enumerate(["out_node", "out_inst", "out_svc"])
    ]

    with tile.TileContext(nc) as tc:
        with (
            tc.tile_pool(name="const", bufs=1) as const,
            tc.tile_pool(name="g", bufs=4) as gpool,
            tc.tile_pool(name="st", bufs=10) as stpool,
            tc.tile_pool(name="evac", bufs=3) as evac,
            tc.tile_pool(name="osb", bufs=4) as opool,
            tc.tile_pool(name="psA", bufs=4, space="PSUM") as psA,
            tc.tile_pool(name="psO", bufs=3, space="PSUM") as psO,
        ):
            # load the leading idx slice first so gathers start ASAP
            idx_t = const.tile([128, nidx_tot // 16], mybir.dt.int16)
            c0 = min(2 * CHUNK * BLK // 16, nidx_tot // 16)
            nc.sync.dma_start(idx_t[:, :c0], idx_d.ap()[:, :c0])
            dA_t = const.tile([128, nblk_tot], f32)
            nc.sync.dma_start(dA_t[:], dA_d.ap())
            dB_t = const.tile([128, nblk_tot], f32)
            nc.sync.dma_start(dB_t[:], dB_d.ap())
            rA_t = const.tile([128, nblk_tot], f32)
            nc.sync.dma_start(rA_t[:], rA_d.ap())
            rB_t = const.tile([128, nblk_tot], f32)
            nc.sync.dma_start(rB_t[:], rB_d.ap())
            ior_t = const.tile([128, kgmax * GW], f16)
            nc.sync.dma_start(ior_t[:], ior_d.ap())
            W_t = const.tile([128, 3 * HID], f32)
            nc.sync.dma_start(W_t[:], W_d.ap())
            b_t = const.tile([128, 3], f32)
            nc.sync.dma_start(b_t[:], b_d.ap())
            if c0 < nidx_tot // 16:
                nc.sync.dma_start(idx_t[:, c0:], idx_d.ap()[:, c0:])

            dma_sem = nc.alloc_semaphore("gdma")
            g_tiles = {}    # global chunk -> gather tile
            st_tiles = {}   # (block, lane, kg) -> one-hot [128, GW]

            def issue_gather(ci, rel, rel_cblk0, rel_blk0, rel_nblk, rel_nslot):
                local_b0 = (ci - rel_cblk0) * CHUNK
                cblk = min(CHUNK, rel_nblk - local_b0)
                gt = gpool.tile([128, CHUNK, LANES * HID], f16, tag="g")
                nidx = cblk * BLK
                reg = nidx
                off16 = (rel_blk0 + local_b0) * BLK // 16
                in_ap = tbl_d[rel].ap()
                in_ap.ap[0] = [HID, meta["tbl_rows"][rel] - 1]
                in_ap.ap[1] = [1, LANES * HID]
                nc.gpsimd.dma_gather(
                    out_ap=gt[:, :cblk, :],
                    in_ap=in_ap,
                    idxs_ap=idx_t[:, off16:off16 + nidx // 16],
                    num_idxs=nidx,
                    num_idxs_reg=reg,
                    elem_size=LANES * HID,
                    elem_step=HID,
                    single_packet=False,
                )
                g_tiles[ci] = gt

            def issue_st(gb, lane, kg, dl_t, rs_t):
                # value-weighted one-hot: rs_dst * (dl == iota), one DVE op in
                # 4x_2p mode (fp16 packed in/out; f32 scalar APs are exempt)
                st = stpool.tile([128, GW], f16, tag="st")
                nc.vector.tensor_scalar(
                    st[:], ior_t[:, kg * GW:(kg + 1) * GW],
                    dl_t[:, gb:gb + 1], rs_t[:, gb:gb + 1],
                    mybir.AluOpType.is_equal, mybir.AluOpType.mult)
                st_tiles[(gb, lane, kg)] = st

            cblk_base = 0
            blk_base = 0
            for rel in range(3):
                ngrp = meta["ngrps"][rel]
                nblk = meta["nblks"][rel]
                nslot = meta["nslots"][rel]
                bstart = meta["bstarts"][rel]
                bend = meta["bends"][rel]
                G0 = meta["G0s"][rel]
                actA = meta["activeA"][rel]
                actB = meta["activeB"][rel]
                for g in range(ngrp):
                    gbs = int(bstart[g * TP])
                    gbe = int(bend[(g + 1) * TP - 1])
                    ems = []
                    for b in range(gbs, gbe):
                        if actA[g, b]:
                            ems.append((b, 0))
                        if actB[g, b]:
                            ems.append((b, 1))
                    agg = psA.tile([128, GW], f32, tag="agg")
                    for i, (b, lane) in enumerate(ems):
                        gb = blk_base + b
                        ci = cblk_base + b // CHUNK
                        if ci not in g_tiles:
                            issue_gather(ci, rel, cblk_base, blk_base,
                                         nblk, nslot)
                        kg = g - int(G0[b])
                        if (gb, lane, kg) not in st_tiles:
                            issue_st(gb, lane, kg,
                                     dA_t if lane == 0 else dB_t,
                                     rA_t if lane == 0 else rB_t)
                        cj = b % CHUNK
                        nc.tensor.matmul(
                            agg[:],
                            g_tiles[ci][:, cj, lane * HID:(lane + 1) * HID],
                            st_tiles[(gb, lane, kg)][:],
                            start=(i == 0), stop=(i == len(ems) - 1))
                    # epilogue: po[h, d] = W.T @ agg; Lrelu(po + b[h])
                    aggsb = evac.tile([128, GW], f32, tag="evac")
                    nc.scalar.copy(aggsb[:], agg[:])
                    po = psO.tile([128, GW], f32, tag="po")
                    nc.tensor.matmul(
                        po[:], W_t[:, rel * HID:(rel + 1) * HID], aggsb[:],
                        start=True, stop=True)
                    og = g % (OUT_GRP // TP)
                    if og == 0:
                        osb = opool.tile([128, OUT_GRP * 128], f16, tag="osb")
                        osb_g0 = g
                    nc.scalar.activation(
                        osb[:, og * GW:(og + 1) * GW], po[:], act_fn,
                        bias=b_t[:, rel:rel + 1], scale=1.0, alpha=0.01)
                    if og == OUT_GRP // TP - 1 or g == ngrp - 1:
                        cols = (g - osb_g0 + 1) * GW
                        dst = out_d[rel].ap()[:, osb_g0 * GW:
                                              osb_g0 * GW + cols]
                        nc.sync.dma_start(dst, osb[:, :cols])
                cblk_base += _cdiv(nblk, CHUNK)
                blk_base += nblk

    nc.compile()
    return nc


def _run(nc, in_maps, trace=False, **kw):
    from concourse import bass_utils
    res = bass_utils.run_bass_kernel_spmd(
        nc, in_maps, core_ids=list(range(NCORES)), trace=trace, **kw)
    return res


def _assemble(results, meta):
    out = np.empty((NODE_N + INST_N + SVC_N, HID), np.float32)
    offs = [0, NODE_N, NODE_N + INST_N]
    names = ["out_node", "out_inst", "out_svc"]
    for rel in range(3):
        D, n_dst = meta["Ds"][rel], meta["n_dsts"][rel]
        ntiles = meta["ntiles"][rel]
        for c in range(NCORES):
            lo = c * D
            n = max(0, min(D, n_dst - lo))
            if n > 0:
                arr = results[c][names[rel]]  # [128 h, ntiles*128 d] fp16
                rows = np.ascontiguousarray(
                    arr.reshape(128, ntiles, 128).transpose(1, 2, 0)
                ).reshape(-1, HID)[:n].astype(np.float32)
                out[offs[rel] + lo: offs[rel] + lo + n] = rows
    return out


def kernel(**inputs):
    import hashlib
    key = "prog"
    h = hashlib.sha1()
    for k in ("sc_src", "sc_dst", "in_src", "in_dst", "ni_src", "ni_dst"):
        h.update(np.ascontiguousarray(np.asarray(inputs[k], np.int32)).tobytes())
    sig = h.hexdigest()
    meta, in_maps = _build_host_data(inputs)
    if key in _cache and _cache[key][0] == sig:
        _, nc, _ = _cache[key]
    else:
        nc = _build_program(meta)
        _cache[key] = (sig, nc, meta)
    res = _run(nc, in_maps)
    return _assemble(res.results, meta)


# revision 19
# speedup vs baseline: 1.3619x; 1.0768x over previous
"""Trainium2 Bass kernel for a heterogeneous GraphConv layer (3 relations).

out = concat([leaky(GC(inst_feat, W_inst, in_*)),     # -> node   (10000)
              leaky(GC(node_feat, W_node, ni_*)),     # -> inst   (100000)
              leaky(GC(svc_feat,  W_svc,  sc_*))])    # -> svc    (20000)

GC(f, W, src, dst) = rsqrt(deg_d) * segsum_dst((rsqrt(deg_s)*f)[src]) @ W + b
(aggregation commutes with the dense @W, so we gather *raw scaled features*
and apply W once per destination tile group).

Strategy: destination-sharded across 8 NeuronCores.  The per-core source
tables are PERMUTED so that rows co-used by the same dst tile sit adjacently;
each dma_gather descriptor then uses an overlapping 512B window (elem 256
fp16 elems, step 128) that fetches TWO consecutive rows — one descriptor
serves up to two edges (lanes A/B).  Descriptor cost on TRN2 is identical
for 256B and 512B payloads, so pairing halves gather DMA time.  Gathers use
prepare_only + trigger_dma so descriptor generation (Pool) pipelines with
the DMA transfers instead of holding Pool.SEQ through them.

Edges (sorted by dst) are packed densely into 128-slot blocks with per-tile
slot quotas (max over cores) so the block->tile map is identical on every
core.  Aggregation runs per GROUP of TP=2 dst tiles (256 PSUM columns):
per (block, lane, group) one DVE tensor_scalar builds a value-weighted
one-hot S[slot, d] = rs_dst * (dl == iota+off) (4x_2p DVE mode; the rsqrt
deg_d scale rides the one-hot so the epilogue needs no rank-1 bias matmul),
and PE accumulates agg[f, d] += G_lane.T @ S in PSUM.  Per group: one
matmul po[h, d] = W.T @ agg, one ScalarE Lrelu(po + b[h]) (bias per
partition in the [h, d] orientation), fp16 output DMA in the transposed
[h, d] layout (the host de-transposes and converts).
"""

import os as _os
from collections import defaultdict

import numpy as np

SVC_N, INST_N, NODE_N, HID = 20000, 100000, 10000, 128
NCORES = 8
BLK = 128           # slots per block (= PE contraction dim)
LANES = 2           # table rows per gather window (512B / 256B fp16 rows)
TP = 2              # dst tiles per aggregation group (256 PSUM columns)
CHUNK = int(_os.environ.get("GNN_CHUNK", "32"))   # blocks per gather instr
OUT_GRP = int(_os.environ.get("GNN_OUT_GRP", "8"))  # dst tiles per out DMA
ACT_MODE = "lrelu"

_cache = {}


def _cdiv(a, b):
    return (a + b - 1) // b


def _rup(a, b):
    return _cdiv(a, b) * b


def _sequence_sources(es, tile):
    """Order this core's used sources so same-tileset sources are adjacent."""
    n = len(es)
    starts = np.flatnonzero(np.r_[True, es[1:] != es[:-1]])
    ends = np.r_[starts[1:], n]
    keys = [tuple(tile[a:b]) for a, b in zip(starts, ends)]
    order = sorted(range(len(starts)), key=lambda i: keys[i])
    return order, starts, ends


def _prep_relation(src, dst, n_src, n_dst, feat_s, rs_d, compact):
    """Host-side sharding/packing for one relation."""
    src = np.asarray(src, np.int64)
    dst = np.asarray(dst, np.int64)

    D = _rup(_cdiv(n_dst, NCORES), 128)  # dst rows per core (padded)
    ntiles = D // 128
    assert ntiles % TP == 0

    cores = []
    for c in range(NCORES):
        lo = c * D
        m = (dst >= lo) & (dst < lo + D)
        es, ed = src[m], dst[m] - lo
        tl = ed >> 7
        order = np.lexsort((tl, es))
        es, ed, tl = es[order], ed[order], tl[order]

        uorder, starts, ends = _sequence_sources(es, tl)
        srcs_u = es[starts]
        nsrc_u = len(srcs_u)

        pos_of_u = np.empty(nsrc_u, np.int64)
        pos_of_u[uorder] = np.arange(nsrc_u)

        if compact:
            table = feat_s[srcs_u[uorder]]
            n_units = nsrc_u
        else:
            used_mask = np.zeros(n_src, bool)
            used_mask[srcs_u] = True
            perm = np.concatenate([srcs_u[uorder],
                                   np.flatnonzero(~used_mask)])
            table = feat_s[perm]
            n_units = n_src

        # slots per tile via the path-greedy pairing over table positions
        slot_k = [[] for _ in range(ntiles)]
        slot_dA = [[] for _ in range(ntiles)]
        slot_dB = [[] for _ in range(ntiles)]
        per_tile = defaultdict(list)  # tile -> list of (pos, [dst_locals])
        for ui in range(nsrc_u):
            a, b = starts[ui], ends[ui]
            p = pos_of_u[ui]
            t0 = a
            while t0 < b:
                t1 = t0
                while t1 < b and tl[t1] == tl[t0]:
                    t1 += 1
                per_tile[tl[t0]].append((p, ed[t0:t1]))
                t0 = t1
        for t, lst in per_tile.items():
            lst.sort(key=lambda x: x[0])
            sk, sa, sb = slot_k[t], slot_dA[t], slot_dB[t]
            prev_pos = -10
            prev_ds = []
            for p, ds in lst:
                ds = list(ds)
                if p == prev_pos + 1 and prev_ds:
                    npair = min(len(prev_ds), len(ds))
                    for i in range(npair):
                        sk.append(prev_pos)
                        sa.append(prev_ds[i])
                        sb.append(ds[i])
                    for d in prev_ds[npair:]:
                        sk.append(prev_pos)
                        sa.append(d)
                        sb.append(-1)
                    ds = ds[npair:]
                else:
                    for d in prev_ds:
                        sk.append(prev_pos)
                        sa.append(d)
                        sb.append(-1)
                prev_pos, prev_ds = p, ds
            for d in prev_ds:
                sk.append(prev_pos)
                sa.append(d)
                sb.append(-1)
            # paired slots first so lane-B tails can be skipped
            osort = sorted(range(len(sk)), key=lambda i: sb[i] < 0)
            slot_k[t] = [sk[i] for i in osort]
            slot_dA[t] = [sa[i] for i in osort]
            slot_dB[t] = [sb[i] for i in osort]

        cores.append(dict(slot_k=slot_k, slot_dA=slot_dA, slot_dB=slot_dB,
                          table=table, n_units=n_units))

    # shared per-tile quotas and block map
    quota = np.zeros(ntiles, np.int64)
    for t in range(ntiles):
        quota[t] = max(max(len(cores[c]["slot_k"][t]) for c in range(NCORES)), 1)
    cum = np.concatenate([[0], np.cumsum(quota)])
    nslot = int(cum[-1])
    nslot_pad = _rup(nslot, BLK)
    nblk = nslot_pad // BLK
    bstart = (cum[:-1] // BLK).astype(np.int64)
    bend = np.minimum(-(-cum[1:] // BLK), nblk).astype(np.int64)
    bend = np.maximum(bend, bstart + 1)
    # T0(b): first tile covering block b; span(b): tiles covered
    T0 = np.zeros(nblk, np.int64)
    cur = 0
    for b in range(nblk):
        while bend[cur] <= b:
            cur += 1
        T0[b] = cur
    span = np.ones(nblk, np.int64)
    for t in range(ntiles):
        for b in range(int(bstart[t]), int(bend[t])):
            span[b] = max(span[b], t - T0[b] + 1)

    # per-core dst rsqrt-degree values (0 beyond n_dst)
    rs_core = []
    for c in range(NCORES):
        lo = c * D
        v = np.zeros(D, np.float32)
        n = max(0, min(D, n_dst - lo))
        if n > 0:
            v[:n] = rs_d[lo:lo + n]
        rs_core.append(v)

    ngrp = ntiles // TP
    activeA = np.zeros((ntiles, nblk), bool)
    activeB = np.zeros((ntiles, nblk), bool)
    for c in range(NCORES):
        d = cores[c]
        kidx = np.zeros(nslot_pad, np.int64)
        dA = np.full(nslot_pad, -1.0, np.float32)
        dB = np.full(nslot_pad, -1.0, np.float32)
        rA = np.zeros(nslot_pad, np.float32)
        rB = np.zeros(nslot_pad, np.float32)
        rsv = rs_core[c]
        for t in range(ntiles):
            off = int(cum[t])
            sk, sa, sb = d["slot_k"][t], d["slot_dA"][t], d["slot_dB"][t]
            for i in range(len(sk)):
                b = (off + i) // BLK
                shift = 128 * int(T0[b])
                kidx[off + i] = sk[i]
                dA[off + i] = sa[i] - shift
                rA[off + i] = rsv[sa[i]]
                activeA[t, b] = True
                if sb[i] >= 0:
                    dB[off + i] = sb[i] - shift
                    rB[off + i] = rsv[sb[i]]
                    activeB[t, b] = True
        # tail pads keep idx 0 (cost model charges num_idxs regardless; a
        # real gather keeps the SBUF block initialized -- NaN x 0 hazard)
        d["kidx"], d["dA"], d["dB"], d["rA"], d["rB"] = kidx, dA, dB, rA, rB
        del d["slot_k"], d["slot_dA"], d["slot_dB"]

    # force one active matmul per tile so every agg gets a start+stop
    for t in range(ntiles):
        if not activeA[t, bstart[t]:bend[t]].any() and \
           not activeB[t, bstart[t]:bend[t]].any():
            activeA[t, bstart[t]] = True

    return dict(cores=cores, ntiles=ntiles, ngrp=ngrp, D=D, n_dst=n_dst,
                nslot=nslot, nslot_pad=nslot_pad, nblk=nblk,
                bstart=bstart, bend=bend, T0=T0, span=span,
                activeA=activeA, activeB=activeB)


def _build_host_data(inputs):
    def prescale(feat, src, n_src):
        deg = np.maximum(np.bincount(np.asarray(src, np.int64),
                                     minlength=n_src), 1.0)
        return (np.asarray(feat, np.float32)
                / np.sqrt(deg)[:, None]).astype(np.float32)

    def rs_of(dstv, n_dst):
        deg = np.maximum(np.bincount(np.asarray(dstv, np.int64),
                                     minlength=n_dst), 1.0)
        return (1.0 / np.sqrt(deg)).astype(np.float32)

    feat0 = prescale(inputs["instance_feat"], inputs["in_src"], INST_N)
    feat1 = prescale(inputs["node_feat"], inputs["ni_src"], NODE_N)
    feat2 = prescale(inputs["svc_feat"], inputs["sc_src"], SVC_N)

    rels = [
        # order matters: output rows are [node_out, inst_out, svc_out]
        _prep_relation(inputs["in_src"], inputs["in_dst"], INST_N, NODE_N,
                       feat0, rs_of(inputs["in_dst"], NODE_N), compact=True),
        _prep_relation(inputs["ni_src"], inputs["ni_dst"], NODE_N, INST_N,
                       feat1, rs_of(inputs["ni_dst"], INST_N), compact=False),
        _prep_relation(inputs["sc_src"], inputs["sc_dst"], SVC_N, SVC_N,
                       feat2, rs_of(inputs["sc_dst"], SVC_N), compact=False),
    ]
    Ws = [inputs["W_inst"], inputs["W_node"], inputs["W_svc"]]
    bs = [inputs["b_inst"], inputs["b_node"], inputs["b_svc"]]

    umax = _rup(max(c["n_units"] for c in rels[0]["cores"]) + 2, 16)
    nblk_tot = sum(r["nblk"] for r in rels)
    nidx_tot = nblk_tot * BLK

    W_cat = np.concatenate([np.asarray(w, np.float32) for w in Ws], axis=1)
    b_col = np.stack([np.asarray(b, np.float32) for b in bs], axis=1)  # [128,3]

    # ramp width: max tile span of any block
    kmax = max(int(r["span"].max()) for r in rels)
    assert kmax * 128 <= 2048, f"ramp {kmax * 128} not fp16-exact"
    iota_ramp = np.tile(np.arange(kmax * 128, dtype=np.float16), (128, 1))

    in_maps = []
    for c in range(NCORES):
        kidx = np.concatenate([r["cores"][c]["kidx"] for r in rels])
        assert kidx.max() < 32768
        idx16 = np.ascontiguousarray(kidx.astype(np.int16).reshape(-1, 16).T)
        idx_sb = np.tile(idx16, (8, 1))

        def blkmaj(name):
            v = np.concatenate([r["cores"][c][name] for r in rels])
            return np.ascontiguousarray(
                v.reshape(nblk_tot, BLK).T).astype(np.float32)

        def mk_tbl(tab, rows):
            out = np.zeros((rows, HID), np.float16)
            out[:len(tab)] = tab.astype(np.float16)
            return np.ascontiguousarray(out)

        in_maps.append({
            "tbl_in": mk_tbl(rels[0]["cores"][c]["table"], umax),
            "tbl_ni": mk_tbl(rels[1]["cores"][c]["table"], NODE_N + 2),
            "tbl_sc": mk_tbl(rels[2]["cores"][c]["table"], SVC_N + 2),
            "idx_sb": np.ascontiguousarray(idx_sb),
            "dA_sb": blkmaj("dA"),
            "dB_sb": blkmaj("dB"),
            "rA_sb": blkmaj("rA"),
            "rB_sb": blkmaj("rB"),
            "W_cat": np.ascontiguousarray(W_cat),
            "b_col": np.ascontiguousarray(b_col),
            "iota_ramp": np.ascontiguousarray(iota_ramp),
        })

    meta = dict(
        umax=umax, nblk_tot=nblk_tot, nidx_tot=nidx_tot, kmax=kmax,
        ntiles=[r["ntiles"] for r in rels],
        ngrps=[r["ngrp"] for r in rels],
        Ds=[r["D"] for r in rels],
        n_dsts=[r["n_dst"] for r in rels],
        nslots=[r["nslot"] for r in rels],
        nblks=[r["nblk"] for r in rels],
        bstarts=[r["bstart"].tolist() for r in rels],
        bends=[r["bend"].tolist() for r in rels],
        T0s=[r["T0"].tolist() for r in rels],
        spans=[r["span"].tolist() for r in rels],
        activeA=[r["activeA"] for r in rels],
        activeB=[r["activeB"] for r in rels],
        tbl_rows=[umax, NODE_N + 2, SVC_N + 2],
    )
    return meta, in_maps


def _build_program(meta):
    import concourse.bacc as bacc
    import concourse.mybir as mybir
    import concourse.tile as tile

    f16 = mybir.dt.float16
    f32 = mybir.dt.float32
    f32r = mybir.dt.float32r
    AF = mybir.ActivationFunctionType
    act_fn = AF.Lrelu if ACT_MODE == "lrelu" else AF.Relu

    nblk_tot, nidx_tot = meta["nblk_tot"], meta["nidx_tot"]
    kmax = meta["kmax"]
    GW = TP * 128  # epilogue group width in dst columns

    nc = bacc.Bacc("TRN2", target_bir_lowering=False, debug=False,
                   enable_asserts=False, num_devices=NCORES)

    tbl_d = [
        nc.dram_tensor(nm, [meta["tbl_rows"][i], HID], f16,
                       kind="ExternalInput")
        for i, nm in enumerate(["tbl_in", "tbl_ni", "tbl_sc"])
    ]
    idx_d = nc.dram_tensor("idx_sb", [128, nidx_tot // 16], mybir.dt.int16,
                           kind="ExternalInput")
    dA_d = nc.dram_tensor("dA_sb", [128, nblk_tot], f32, kind="ExternalInput")
    dB_d = nc.dram_tensor("dB_sb", [128, nblk_tot], f32, kind="ExternalInput")
    rA_d = nc.dram_tensor("rA_sb", [128, nblk_tot], f32, kind="ExternalInput")
    rB_d = nc.dram_tensor("rB_sb", [128, nblk_tot], f32, kind="ExternalInput")
    W_d = nc.dram_tensor("W_cat", [128, 3 * HID], f32, kind="ExternalInput")
    b_d = nc.dram_tensor("b_col", [128, 3], f32, kind="ExternalInput")
    ior_d = nc.dram_tensor("iota_ramp", [128, kmax * 128], f16,
                           kind="ExternalInput")

    out_d = [
        nc.dram_tensor(nm, [128, meta["ntiles"][i] * 128], f16,
                       kind="ExternalOutput")
        for i, nm in enumerate(["out_node", "out_inst", "out_svc"])
    ]

    with tile.TileContext(nc) as tc:
        with (
            tc.tile_pool(name="const", bufs=1) as const,
            tc.tile_pool(name="g", bufs=6) as gpool,
            tc.tile_pool(name="st", bufs=16) as stpool,
            tc.tile_pool(name="evac", bufs=4) as evac,
            tc.tile_pool(name="osb", bufs=4) as opool,
            tc.tile_pool(name="psA", bufs=6, space="PSUM") as psA,
            tc.tile_pool(name="psO", bufs=2, space="PSUM") as psO,
        ):
            # load the leading idx slice first so gathers start ASAP
            idx_t = const.tile([128, nidx_tot // 16], mybir.dt.int16)
            c0 = min(2 * CHUNK * BLK // 16, nidx_tot // 16)
            nc.sync.dma_start(idx_t[:, :c0], idx_d.ap()[:, :c0])
            dA_t = const.tile([128, nblk_tot], f32)
            nc.sync.dma_start(dA_t[:], dA_d.ap())
            dB_t = const.tile([128, nblk_tot], f32)
            nc.sync.dma_start(dB_t[:], dB_d.ap())
            rA_t = const.tile([128, nblk_tot], f32)
            nc.sync.dma_start(rA_t[:], rA_d.ap())
            rB_t = const.tile([128, nblk_tot], f32)
            nc.sync.dma_start(rB_t[:], rB_d.ap())
            ior_t = const.tile([128, kmax * 128], f16)
            nc.sync.dma_start(ior_t[:], ior_d.ap())
            W_t = const.tile([128, 3 * HID], f32)
            nc.sync.dma_start(W_t[:], W_d.ap())
            W_r = const.tile([128, 3 * HID], f32r)
            nc.scalar.copy(W_r[:], W_t[:])
            b_t = const.tile([128, 3], f32)
            nc.sync.dma_start(b_t[:], b_d.ap())
            if c0 < nidx_tot // 16:
                nc.sync.dma_start(idx_t[:, c0:], idx_d.ap()[:, c0:])

            dma_sem = nc.alloc_semaphore("gdma")
            g_tiles = {}    # global chunk -> gather tile
            st_tiles = {}   # (block, lane, kg) -> one-hot [128, GW]

            def issue_gather(ci, rel, rel_cblk0, rel_blk0, rel_nblk, rel_nslot):
                local_b0 = (ci - rel_cblk0) * CHUNK
                cblk = min(CHUNK, rel_nblk - local_b0)
                gt = gpool.tile([128, CHUNK, LANES * HID], f16, tag="g")
                nidx = cblk * BLK
                reg = nidx
                off16 = (rel_blk0 + local_b0) * BLK // 16
                in_ap = tbl_d[rel].ap()
                in_ap.ap[0] = [HID, meta["tbl_rows"][rel] - 1]
                in_ap.ap[1] = [1, LANES * HID]
                nc.gpsimd.dma_gather(
                    out_ap=gt[:, :cblk, :],
                    in_ap=in_ap,
                    idxs_ap=idx_t[:, off16:off16 + nidx // 16],
                    num_idxs=nidx,
                    num_idxs_reg=reg,
                    elem_size=LANES * HID,
                    elem_step=HID,
                    single_packet=False,
                )
                g_tiles[ci] = gt

            def issue_st(gb, lane, wid, dl_t, rs_t):
                # value-weighted one-hot: rs_dst * (dl == iota), one DVE op in
                # 4x_2p mode (fp16 packed in/out; f32 scalar APs are exempt).
                # Built once per (block, lane) covering the block's full tile
                # span; per-tile matmuls slice 128-column windows from it.
                st = stpool.tile([128, kmax * 128], f16, tag="st")
                nc.vector.tensor_scalar(
                    st[:, :wid], ior_t[:, :wid],
                    dl_t[:, gb:gb + 1], rs_t[:, gb:gb + 1],
                    mybir.AluOpType.is_equal, mybir.AluOpType.mult)
                st_tiles[(gb, lane)] = st

            cblk_base = 0
            blk_base = 0
            for rel in range(3):
                ngrp = meta["ngrps"][rel]
                nblk = meta["nblks"][rel]
                nslot = meta["nslots"][rel]
                bstart = meta["bstarts"][rel]
                bend = meta["bends"][rel]
                T0 = meta["T0s"][rel]
                span = meta["spans"][rel]
                actA = meta["activeA"][rel]
                actB = meta["activeB"][rel]
                for g in range(ngrp):
                    aggs = []
                    for kt in range(TP):
                        t = g * TP + kt
                        ems = []
                        for b in range(int(bstart[t]), int(bend[t])):
                            if actA[t, b]:
                                ems.append((b, 0))
                            if actB[t, b]:
                                ems.append((b, 1))
                        agg = psA.tile([128, 128], f32, tag="agg")
                        for i, (b, lane) in enumerate(ems):
                            gb = blk_base + b
                            ci = cblk_base + b // CHUNK
                            if ci not in g_tiles:
                                issue_gather(ci, rel, cblk_base, blk_base,
                                             nblk, nslot)
                            if (gb, lane) not in st_tiles:
                                issue_st(gb, lane, int(span[b]) * 128,
                                         dA_t if lane == 0 else dB_t,
                                         rA_t if lane == 0 else rB_t)
                            k = t - int(T0[b])
                            cj = b % CHUNK
                            nc.tensor.matmul(
                                agg[:],
                                g_tiles[ci][:, cj, lane * HID:(lane + 1) * HID],
                                st_tiles[(gb, lane)][:, k * 128:(k + 1) * 128],
                                start=(i == 0), stop=(i == len(ems) - 1))
                        aggs.append(agg)
                    # epilogue: po[h, d] = W.T @ agg_pair; Lrelu(po + b[h])
                    aggsb = evac.tile([128, GW], f32r, tag="evac")
                    for kt in range(TP):
                        nc.scalar.copy(aggsb[:, kt * 128:(kt + 1) * 128],
                                       aggs[kt][:])
                    po = psO.tile([128, GW], f32, tag="po")
                    nc.tensor.matmul(
                        po[:],
                        W_r[:, rel * HID:(rel + 1) * HID],
                        aggsb[:],
                        start=True, stop=True)
                    og = g % (OUT_GRP // TP)
                    if og == 0:
                        osb = opool.tile([128, OUT_GRP * 128], f16, tag="osb")
                        osb_g0 = g
                    nc.scalar.activation(
                        osb[:, og * GW:(og + 1) * GW], po[:], act_fn,
                        bias=b_t[:, rel:rel + 1], scale=1.0, alpha=0.01)
                    if og == OUT_GRP // TP - 1 or g == ngrp - 1:
                        cols = (g - osb_g0 + 1) * GW
                        dst = out_d[rel].ap()[:, osb_g0 * GW:
                                              osb_g0 * GW + cols]
                        nc.sync.dma_start(dst, osb[:, :cols])
                cblk_base += _cdiv(nblk, CHUNK)
                blk_base += nblk

    nc.compile()
    return nc


def _run(nc, in_maps, trace=False, **kw):
    from concourse import bass_utils
    res = bass_utils.run_bass_kernel_spmd(
        nc, in_maps, core_ids=list(range(NCORES)), trace=trace, **kw)
    return res


def _assemble(results, meta):
    out = np.empty((NODE_N + INST_N + SVC_N, HID), np.float32)
    offs = [0, NODE_N, NODE_N + INST_N]
    names = ["out_node", "out_inst", "out_svc"]
    for rel in range(3):
        D, n_dst = meta["Ds"][rel], meta["n_dsts"][rel]
        ntiles = meta["ntiles"][rel]
        for c in range(NCORES):
            lo = c * D
            n = max(0, min(D, n_dst - lo))
            if n > 0:
                arr = results[c][names[rel]]  # [128 h, ntiles*128 d] fp16
                rows = np.ascontiguousarray(
                    arr.reshape(128, ntiles, 128).transpose(1, 2, 0)
                ).reshape(-1, HID)[:n].astype(np.float32)
                out[offs[rel] + lo: offs[rel] + lo + n] = rows
    return out


def kernel(**inputs):
    import hashlib
    key = "prog"
    h = hashlib.sha1()
    for k in ("sc_src", "sc_dst", "in_src", "in_dst", "ni_src", "ni_dst"):
        h.update(np.ascontiguousarray(np.asarray(inputs[k], np.int32)).tobytes())
    sig = h.hexdigest()
    meta, in_maps = _build_host_data(inputs)
    if key in _cache and _cache[key][0] == sig:
        _, nc, _ = _cache[key]
    else:
        nc = _build_program(meta)
        _cache[key] = (sig, nc, meta)
    res = _run(nc, in_maps)
    return _assemble(res.results, meta)


# revision 22
# speedup vs baseline: 1.3948x; 1.0241x over previous
"""Trainium2 Bass kernel for a heterogeneous GraphConv layer (3 relations).

out = concat([leaky(GC(inst_feat, W_inst, in_*)),     # -> node   (10000)
              leaky(GC(node_feat, W_node, ni_*)),     # -> inst   (100000)
              leaky(GC(svc_feat,  W_svc,  sc_*))])    # -> svc    (20000)

GC(f, W, src, dst) = rsqrt(deg_d) * segsum_dst((rsqrt(deg_s)*f)[src]) @ W + b
(aggregation commutes with the dense @W, so we gather *raw scaled features*
and apply W once per destination tile group).

Strategy: destination-sharded across 8 NeuronCores.  The per-core source
tables are PERMUTED so that rows co-used by the same dst tile sit adjacently;
each dma_gather descriptor then uses an overlapping 512B window (elem 256
fp16 elems, step 128) that fetches TWO consecutive rows — one descriptor
serves up to two edges (lanes A/B).  Descriptor cost on TRN2 is identical
for 256B and 512B payloads, so pairing halves gather DMA time.  Gathers use
prepare_only + trigger_dma so descriptor generation (Pool) pipelines with
the DMA transfers instead of holding Pool.SEQ through them.

Edges (sorted by dst) are packed densely into 128-slot blocks with per-tile
slot quotas (max over cores) so the block->tile map is identical on every
core.  Aggregation runs per GROUP of TP=2 dst tiles (256 PSUM columns):
per (block, lane, group) one DVE tensor_scalar builds a value-weighted
one-hot S[slot, d] = rs_dst * (dl == iota+off) (4x_2p DVE mode; the rsqrt
deg_d scale rides the one-hot so the epilogue needs no rank-1 bias matmul),
and PE accumulates agg[f, d] += G_lane.T @ S in PSUM.  Per group: one
matmul po[h, d] = W.T @ agg, one ScalarE Lrelu(po + b[h]) (bias per
partition in the [h, d] orientation), fp16 output DMA in the transposed
[h, d] layout (the host de-transposes and converts).
"""

import os as _os
from collections import defaultdict

import numpy as np

SVC_N, INST_N, NODE_N, HID = 20000, 100000, 10000, 128
NCORES = 8
BLK = 128           # slots per block (= PE contraction dim)
LANES = 2           # table rows per gather window (512B / 256B fp16 rows)
TP = 2              # dst tiles per aggregation group (256 PSUM columns)
CHUNK = int(_os.environ.get("GNN_CHUNK", "32"))   # blocks per gather instr
PREP = bool(int(_os.environ.get("GNN_PREP", "0")))  # prepare_only + trigger
OUT_GRP = int(_os.environ.get("GNN_OUT_GRP", "8"))  # dst tiles per out DMA
ACT_MODE = "lrelu"

_cache = {}


def _cdiv(a, b):
    return (a + b - 1) // b


def _rup(a, b):
    return _cdiv(a, b) * b


def _sequence_sources(es, tile):
    """Order this core's used sources so same-tileset sources are adjacent."""
    n = len(es)
    starts = np.flatnonzero(np.r_[True, es[1:] != es[:-1]])
    ends = np.r_[starts[1:], n]
    keys = [tuple(tile[a:b]) for a, b in zip(starts, ends)]
    order = sorted(range(len(starts)), key=lambda i: keys[i])
    return order, starts, ends


def _prep_relation(src, dst, n_src, n_dst, feat_s, rs_d, compact):
    """Host-side sharding/packing for one relation."""
    src = np.asarray(src, np.int64)
    dst = np.asarray(dst, np.int64)

    D = _rup(_cdiv(n_dst, NCORES), 128)  # dst rows per core (padded)
    ntiles = D // 128
    assert ntiles % TP == 0

    cores = []
    for c in range(NCORES):
        lo = c * D
        m = (dst >= lo) & (dst < lo + D)
        es, ed = src[m], dst[m] - lo
        tl = ed >> 7
        order = np.lexsort((tl, es))
        es, ed, tl = es[order], ed[order], tl[order]

        uorder, starts, ends = _sequence_sources(es, tl)
        srcs_u = es[starts]
        nsrc_u = len(srcs_u)

        pos_of_u = np.empty(nsrc_u, np.int64)
        pos_of_u[uorder] = np.arange(nsrc_u)

        if compact:
            table = feat_s[srcs_u[uorder]]
            n_units = nsrc_u
        else:
            used_mask = np.zeros(n_src, bool)
            used_mask[srcs_u] = True
            perm = np.concatenate([srcs_u[uorder],
                                   np.flatnonzero(~used_mask)])
            table = feat_s[perm]
            n_units = n_src

        # slots per tile via the path-greedy pairing over table positions
        slot_k = [[] for _ in range(ntiles)]
        slot_dA = [[] for _ in range(ntiles)]
        slot_dB = [[] for _ in range(ntiles)]
        per_tile = defaultdict(list)  # tile -> list of (pos, [dst_locals])
        for ui in range(nsrc_u):
            a, b = starts[ui], ends[ui]
            p = pos_of_u[ui]
            t0 = a
            while t0 < b:
                t1 = t0
                while t1 < b and tl[t1] == tl[t0]:
                    t1 += 1
                per_tile[tl[t0]].append((p, ed[t0:t1]))
                t0 = t1
        for t, lst in per_tile.items():
            lst.sort(key=lambda x: x[0])
            sk, sa, sb = slot_k[t], slot_dA[t], slot_dB[t]
            prev_pos = -10
            prev_ds = []
            for p, ds in lst:
                ds = list(ds)
                if p == prev_pos + 1 and prev_ds:
                    npair = min(len(prev_ds), len(ds))
                    for i in range(npair):
                        sk.append(prev_pos)
                        sa.append(prev_ds[i])
                        sb.append(ds[i])
                    for d in prev_ds[npair:]:
                        sk.append(prev_pos)
                        sa.append(d)
                        sb.append(-1)
                    ds = ds[npair:]
                else:
                    for d in prev_ds:
                        sk.append(prev_pos)
                        sa.append(d)
                        sb.append(-1)
                prev_pos, prev_ds = p, ds
            for d in prev_ds:
                sk.append(prev_pos)
                sa.append(d)
                sb.append(-1)
            # paired slots first so lane-B tails can be skipped
            osort = sorted(range(len(sk)), key=lambda i: sb[i] < 0)
            slot_k[t] = [sk[i] for i in osort]
            slot_dA[t] = [sa[i] for i in osort]
            slot_dB[t] = [sb[i] for i in osort]

        cores.append(dict(slot_k=slot_k, slot_dA=slot_dA, slot_dB=slot_dB,
                          table=table, n_units=n_units))

    # shared per-tile quotas and block map
    quota = np.zeros(ntiles, np.int64)
    for t in range(ntiles):
        quota[t] = max(max(len(cores[c]["slot_k"][t]) for c in range(NCORES)), 1)
    cum = np.concatenate([[0], np.cumsum(quota)])
    nslot = int(cum[-1])
    nslot_pad = _rup(nslot, BLK)
    nblk = nslot_pad // BLK
    bstart = (cum[:-1] // BLK).astype(np.int64)
    bend = np.minimum(-(-cum[1:] // BLK), nblk).astype(np.int64)
    bend = np.maximum(bend, bstart + 1)
    # T0(b): first tile covering block b; span(b): tiles covered
    T0 = np.zeros(nblk, np.int64)
    cur = 0
    for b in range(nblk):
        while bend[cur] <= b:
            cur += 1
        T0[b] = cur
    span = np.ones(nblk, np.int64)
    for t in range(ntiles):
        for b in range(int(bstart[t]), int(bend[t])):
            span[b] = max(span[b], t - T0[b] + 1)

    # per-core dst rsqrt-degree values (0 beyond n_dst)
    rs_core = []
    for c in range(NCORES):
        lo = c * D
        v = np.zeros(D, np.float32)
        n = max(0, min(D, n_dst - lo))
        if n > 0:
            v[:n] = rs_d[lo:lo + n]
        rs_core.append(v)

    ngrp = ntiles // TP
    activeA = np.zeros((ntiles, nblk), bool)
    activeB = np.zeros((ntiles, nblk), bool)
    for c in range(NCORES):
        d = cores[c]
        kidx = np.zeros(nslot_pad, np.int64)
        dA = np.full(nslot_pad, -1.0, np.float32)
        dB = np.full(nslot_pad, -1.0, np.float32)
        rA = np.zeros(nslot_pad, np.float32)
        rB = np.zeros(nslot_pad, np.float32)
        rsv = rs_core[c]
        for t in range(ntiles):
            off = int(cum[t])
            sk, sa, sb = d["slot_k"][t], d["slot_dA"][t], d["slot_dB"][t]
            for i in range(len(sk)):
                b = (off + i) // BLK
                shift = 128 * int(T0[b])
                kidx[off + i] = sk[i]
                dA[off + i] = sa[i] - shift
                rA[off + i] = rsv[sa[i]]
                activeA[t, b] = True
                if sb[i] >= 0:
                    dB[off + i] = sb[i] - shift
                    rB[off + i] = rsv[sb[i]]
                    activeB[t, b] = True
        # tail pads keep idx 0 (cost model charges num_idxs regardless; a
        # real gather keeps the SBUF block initialized -- NaN x 0 hazard)
        d["kidx"], d["dA"], d["dB"], d["rA"], d["rB"] = kidx, dA, dB, rA, rB
        del d["slot_k"], d["slot_dA"], d["slot_dB"]

    # force one active matmul per tile so every agg gets a start+stop
    for t in range(ntiles):
        if not activeA[t, bstart[t]:bend[t]].any() and \
           not activeB[t, bstart[t]:bend[t]].any():
            activeA[t, bstart[t]] = True

    return dict(cores=cores, ntiles=ntiles, ngrp=ngrp, D=D, n_dst=n_dst,
                nslot=nslot, nslot_pad=nslot_pad, nblk=nblk,
                bstart=bstart, bend=bend, T0=T0, span=span,
                activeA=activeA, activeB=activeB)


def _build_host_data(inputs):
    def prescale(feat, src, n_src):
        deg = np.maximum(np.bincount(np.asarray(src, np.int64),
                                     minlength=n_src), 1.0)
        return (np.asarray(feat, np.float32)
                / np.sqrt(deg)[:, None]).astype(np.float32)

    def rs_of(dstv, n_dst):
        deg = np.maximum(np.bincount(np.asarray(dstv, np.int64),
                                     minlength=n_dst), 1.0)
        return (1.0 / np.sqrt(deg)).astype(np.float32)

    feat0 = prescale(inputs["instance_feat"], inputs["in_src"], INST_N)
    feat1 = prescale(inputs["node_feat"], inputs["ni_src"], NODE_N)
    feat2 = prescale(inputs["svc_feat"], inputs["sc_src"], SVC_N)

    rels = [
        # order matters: output rows are [node_out, inst_out, svc_out]
        _prep_relation(inputs["in_src"], inputs["in_dst"], INST_N, NODE_N,
                       feat0, rs_of(inputs["in_dst"], NODE_N), compact=True),
        _prep_relation(inputs["ni_src"], inputs["ni_dst"], NODE_N, INST_N,
                       feat1, rs_of(inputs["ni_dst"], INST_N), compact=False),
        _prep_relation(inputs["sc_src"], inputs["sc_dst"], SVC_N, SVC_N,
                       feat2, rs_of(inputs["sc_dst"], SVC_N), compact=False),
    ]
    Ws = [inputs["W_inst"], inputs["W_node"], inputs["W_svc"]]
    bs = [inputs["b_inst"], inputs["b_node"], inputs["b_svc"]]

    umax = _rup(max(c["n_units"] for c in rels[0]["cores"]) + 2, 16)
    nblk_tot = sum(r["nblk"] for r in rels)
    nidx_tot = nblk_tot * BLK

    W_cat = np.concatenate([np.asarray(w, np.float32) for w in Ws], axis=1)
    b_col = np.stack([np.asarray(b, np.float32) for b in bs], axis=1)  # [128,3]

    # ramp width: max tile span of any block
    kmax = max(int(r["span"].max()) for r in rels)
    assert kmax * 128 <= 2048, f"ramp {kmax * 128} not fp16-exact"
    iota_ramp = np.tile(np.arange(kmax * 128, dtype=np.float16), (128, 1))

    in_maps = []
    for c in range(NCORES):
        kidx = np.concatenate([r["cores"][c]["kidx"] for r in rels])
        assert kidx.max() < 32768
        idx16 = np.ascontiguousarray(kidx.astype(np.int16).reshape(-1, 16).T)
        idx_sb = np.tile(idx16, (8, 1))

        def blkmaj(name):
            v = np.concatenate([r["cores"][c][name] for r in rels])
            return np.ascontiguousarray(
                v.reshape(nblk_tot, BLK).T).astype(np.float32)

        def mk_tbl(tab, rows):
            out = np.zeros((rows, HID), np.float16)
            out[:len(tab)] = tab.astype(np.float16)
            return np.ascontiguousarray(out)

        in_maps.append({
            "tbl_in": mk_tbl(rels[0]["cores"][c]["table"], umax),
            "tbl_ni": mk_tbl(rels[1]["cores"][c]["table"], NODE_N + 2),
            "tbl_sc": mk_tbl(rels[2]["cores"][c]["table"], SVC_N + 2),
            "idx_sb": np.ascontiguousarray(idx_sb),
            "dA_sb": blkmaj("dA"),
            "dB_sb": blkmaj("dB"),
            "rA_sb": blkmaj("rA"),
            "rB_sb": blkmaj("rB"),
            "W_cat": np.ascontiguousarray(W_cat),
            "b_col": np.ascontiguousarray(b_col),
            "iota_ramp": np.ascontiguousarray(iota_ramp),
        })

    meta = dict(
        umax=umax, nblk_tot=nblk_tot, nidx_tot=nidx_tot, kmax=kmax,
        ntiles=[r["ntiles"] for r in rels],
        ngrps=[r["ngrp"] for r in rels],
        Ds=[r["D"] for r in rels],
        n_dsts=[r["n_dst"] for r in rels],
        nslots=[r["nslot"] for r in rels],
        nblks=[r["nblk"] for r in rels],
        bstarts=[r["bstart"].tolist() for r in rels],
        bends=[r["bend"].tolist() for r in rels],
        T0s=[r["T0"].tolist() for r in rels],
        spans=[r["span"].tolist() for r in rels],
        activeA=[r["activeA"] for r in rels],
        activeB=[r["activeB"] for r in rels],
        tbl_rows=[umax, NODE_N + 2, SVC_N + 2],
    )
    return meta, in_maps


def _build_program(meta):
    import concourse.bacc as bacc
    import concourse.mybir as mybir
    import concourse.tile as tile

    f16 = mybir.dt.float16
    f32 = mybir.dt.float32
    f32r = mybir.dt.float32r
    AF = mybir.ActivationFunctionType
    act_fn = AF.Lrelu if ACT_MODE == "lrelu" else AF.Relu

    nblk_tot, nidx_tot = meta["nblk_tot"], meta["nidx_tot"]
    kmax = meta["kmax"]
    GW = TP * 128  # epilogue group width in dst columns

    nc = bacc.Bacc("TRN2", target_bir_lowering=False, debug=False,
                   enable_asserts=False, num_devices=NCORES)

    tbl_d = [
        nc.dram_tensor(nm, [meta["tbl_rows"][i], HID], f16,
                       kind="ExternalInput")
        for i, nm in enumerate(["tbl_in", "tbl_ni", "tbl_sc"])
    ]
    idx_d = nc.dram_tensor("idx_sb", [128, nidx_tot // 16], mybir.dt.int16,
                           kind="ExternalInput")
    dA_d = nc.dram_tensor("dA_sb", [128, nblk_tot], f32, kind="ExternalInput")
    dB_d = nc.dram_tensor("dB_sb", [128, nblk_tot], f32, kind="ExternalInput")
    rA_d = nc.dram_tensor("rA_sb", [128, nblk_tot], f32, kind="ExternalInput")
    rB_d = nc.dram_tensor("rB_sb", [128, nblk_tot], f32, kind="ExternalInput")
    W_d = nc.dram_tensor("W_cat", [128, 3 * HID], f32, kind="ExternalInput")
    b_d = nc.dram_tensor("b_col", [128, 3], f32, kind="ExternalInput")
    ior_d = nc.dram_tensor("iota_ramp", [128, kmax * 128], f16,
                           kind="ExternalInput")

    out_d = [
        nc.dram_tensor(nm, [128, meta["ntiles"][i] * 128], f16,
                       kind="ExternalOutput")
        for i, nm in enumerate(["out_node", "out_inst", "out_svc"])
    ]

    with tile.TileContext(nc) as tc:
        with (
            tc.tile_pool(name="const", bufs=1) as const,
            tc.tile_pool(name="g", bufs=3) as gpool,
            tc.tile_pool(name="st", bufs=16) as stpool,
            tc.tile_pool(name="evac", bufs=4) as evac,
            tc.tile_pool(name="osb", bufs=4) as opool,
            tc.tile_pool(name="psA", bufs=6, space="PSUM") as psA,
            tc.tile_pool(name="psO", bufs=2, space="PSUM") as psO,
        ):
            # load the leading idx slice first so gathers start ASAP
            idx_t = const.tile([128, nidx_tot // 16], mybir.dt.int16)
            c0 = min(2 * CHUNK * BLK // 16, nidx_tot // 16)
            nc.sync.dma_start(idx_t[:, :c0], idx_d.ap()[:, :c0])
            dA_t = const.tile([128, nblk_tot], f32)
            nc.sync.dma_start(dA_t[:], dA_d.ap())
            dB_t = const.tile([128, nblk_tot], f32)
            nc.sync.dma_start(dB_t[:], dB_d.ap())
            rA_t = const.tile([128, nblk_tot], f32)
            nc.sync.dma_start(rA_t[:], rA_d.ap())
            rB_t = const.tile([128, nblk_tot], f32)
            nc.sync.dma_start(rB_t[:], rB_d.ap())
            ior_t = const.tile([128, kmax * 128], f16)
            nc.sync.dma_start(ior_t[:], ior_d.ap())
            W_t = const.tile([128, 3 * HID], f32)
            nc.sync.dma_start(W_t[:], W_d.ap())
            W_r = const.tile([128, 3 * HID], f32r)
            nc.scalar.copy(W_r[:], W_t[:])
            b_t = const.tile([128, 3], f32)
            nc.sync.dma_start(b_t[:], b_d.ap())
            if c0 < nidx_tot // 16:
                nc.sync.dma_start(idx_t[:, c0:], idx_d.ap()[:, c0:])

            dma_sems = {}
            g_tiles = {}    # global chunk -> gather tile
            st_tiles = {}   # (block, lane, kg) -> one-hot [128, GW]

            def issue_gather(ci, rel, rel_cblk0, rel_blk0, rel_nblk, rel_nslot):
                local_b0 = (ci - rel_cblk0) * CHUNK
                cblk = min(CHUNK, rel_nblk - local_b0)
                gt = gpool.tile([128, CHUNK, LANES * HID], f16, tag="g")
                nidx = cblk * BLK
                reg = nidx
                off16 = (rel_blk0 + local_b0) * BLK // 16
                in_ap = tbl_d[rel].ap()
                in_ap.ap[0] = [HID, meta["tbl_rows"][rel] - 1]
                in_ap.ap[1] = [1, LANES * HID]
                if PREP:
                    dma_sems[ci] = nc.alloc_semaphore(f"gdma{ci}")
                kw = dict(prepare_only=True, sem=dma_sems[ci]) if PREP else {}
                nc.gpsimd.dma_gather(
                    out_ap=gt[:, :cblk, :],
                    in_ap=in_ap,
                    idxs_ap=idx_t[:, off16:off16 + nidx // 16],
                    num_idxs=nidx,
                    num_idxs_reg=reg,
                    elem_size=LANES * HID,
                    elem_step=HID,
                    single_packet=False,
                    **kw,
                )
                if PREP:
                    nc.gpsimd.trigger_dma(count=None)
                g_tiles[ci] = gt

            def issue_st(gb, lane, wid, dl_t, rs_t):
                # value-weighted one-hot: rs_dst * (dl == iota), one DVE op in
                # 4x_2p mode (fp16 packed in/out; f32 scalar APs are exempt).
                # Built once per (block, lane) covering the block's full tile
                # span; per-tile matmuls slice 128-column windows from it.
                st = stpool.tile([128, kmax * 128], f16, tag="st")
                nc.vector.tensor_scalar(
                    st[:, :wid], ior_t[:, :wid],
                    dl_t[:, gb:gb + 1], rs_t[:, gb:gb + 1],
                    mybir.AluOpType.is_equal, mybir.AluOpType.mult)
                st_tiles[(gb, lane)] = st

            cblk_base = 0
            blk_base = 0
            for rel in range(3):
                ngrp = meta["ngrps"][rel]
                nblk = meta["nblks"][rel]
                nslot = meta["nslots"][rel]
                bstart = meta["bstarts"][rel]
                bend = meta["bends"][rel]
                T0 = meta["T0s"][rel]
                span = meta["spans"][rel]
                actA = meta["activeA"][rel]
                actB = meta["activeB"][rel]
                for g in range(ngrp):
                    aggs = []
                    for kt in range(TP):
                        t = g * TP + kt
                        ems = []
                        for b in range(int(bstart[t]), int(bend[t])):
                            if actA[t, b]:
                                ems.append((b, 0))
                            if actB[t, b]:
                                ems.append((b, 1))
                        agg = psA.tile([128, 128], f32, tag="agg")
                        for i, (b, lane) in enumerate(ems):
                            gb = blk_base + b
                            ci = cblk_base + b // CHUNK
                            if ci not in g_tiles:
                                issue_gather(ci, rel, cblk_base, blk_base,
                                             nblk, nslot)
                            if (gb, lane) not in st_tiles:
                                issue_st(gb, lane, int(span[b]) * 128,
                                         dA_t if lane == 0 else dB_t,
                                         rA_t if lane == 0 else rB_t)
                            k = t - int(T0[b])
                            cj = b % CHUNK
                            nc.tensor.matmul(
                                agg[:],
                                g_tiles[ci][:, cj, lane * HID:(lane + 1) * HID],
                                st_tiles[(gb, lane)][:, k * 128:(k + 1) * 128],
                                start=(i == 0), stop=(i == len(ems) - 1))
                        aggs.append(agg)
                    # epilogue: po[h, d] = W.T @ agg_pair; Lrelu(po + b[h])
                    aggsb = evac.tile([128, GW], f32r, tag="evac")
                    for kt in range(TP):
                        nc.scalar.copy(aggsb[:, kt * 128:(kt + 1) * 128],
                                       aggs[kt][:])
                    po = psO.tile([128, GW], f32, tag="po")
                    nc.tensor.matmul(
                        po[:],
                        W_r[:, rel * HID:(rel + 1) * HID],
                        aggsb[:],
                        start=True, stop=True)
                    og = g % (OUT_GRP // TP)
                    if og == 0:
                        osb = opool.tile([128, OUT_GRP * 128], f16, tag="osb")
                        osb_g0 = g
                    nc.scalar.activation(
                        osb[:, og * GW:(og + 1) * GW], po[:], act_fn,
                        bias=b_t[:, rel:rel + 1], scale=1.0, alpha=0.01)
                    if og == OUT_GRP // TP - 1 or g == ngrp - 1:
                        cols = (g - osb_g0 + 1) * GW
                        dst = out_d[rel].ap()[:, osb_g0 * GW:
                                              osb_g0 * GW + cols]
                        nc.sync.dma_start(dst, osb[:, :cols])
                cblk_base += _cdiv(nblk, CHUNK)
                blk_base += nblk

    nc.compile()
    return nc


def _run(nc, in_maps, trace=False, **kw):
    from concourse import bass_utils
    res = bass_utils.run_bass_kernel_spmd(
        nc, in_maps, core_ids=list(range(NCORES)), trace=trace, **kw)
    return res


def _assemble(results, meta):
    out = np.empty((NODE_N + INST_N + SVC_N, HID), np.float32)
    offs = [0, NODE_N, NODE_N + INST_N]
    names = ["out_node", "out_inst", "out_svc"]
    for rel in range(3):
        D, n_dst = meta["Ds"][rel], meta["n_dsts"][rel]
        ntiles = meta["ntiles"][rel]
        for c in range(NCORES):
            lo = c * D
            n = max(0, min(D, n_dst - lo))
            if n > 0:
                arr = results[c][names[rel]]  # [128 h, ntiles*128 d] fp16
                rows = np.ascontiguousarray(
                    arr.reshape(128, ntiles, 128).transpose(1, 2, 0)
                ).reshape(-1, HID)[:n].astype(np.float32)
                out[offs[rel] + lo: offs[rel] + lo + n] = rows
    return out


def kernel(**inputs):
    import hashlib
    key = "prog"
    h = hashlib.sha1()
    for k in ("sc_src", "sc_dst", "in_src", "in_dst", "ni_src", "ni_dst"):
        h.update(np.ascontiguousarray(np.asarray(inputs[k], np.int32)).tobytes())
    sig = h.hexdigest()
    meta, in_maps = _build_host_data(inputs)
    if key in _cache and _cache[key][0] == sig:
        _, nc, _ = _cache[key]
    else:
        nc = _build_program(meta)
        _cache[key] = (sig, nc, meta)
    res = _run(nc, in_maps)
    return _assemble(res.results, meta)


# revision 24
# speedup vs baseline: 1.3990x; 1.0030x over previous
"""Trainium2 Bass kernel for a heterogeneous GraphConv layer (3 relations).

out = concat([leaky(GC(inst_feat, W_inst, in_*)),     # -> node   (10000)
              leaky(GC(node_feat, W_node, ni_*)),     # -> inst   (100000)
              leaky(GC(svc_feat,  W_svc,  sc_*))])    # -> svc    (20000)

GC(f, W, src, dst) = rsqrt(deg_d) * segsum_dst((rsqrt(deg_s)*f)[src]) @ W + b
(aggregation commutes with the dense @W, so we gather *raw scaled features*
and apply W once per destination tile group).

Strategy: destination-sharded across 8 NeuronCores.  The per-core source
tables are PERMUTED so that rows co-used by the same dst tile sit adjacently;
each dma_gather descriptor then uses an overlapping 512B window (elem 256
fp16 elems, step 128) that fetches TWO consecutive rows — one descriptor
serves up to two edges (lanes A/B).  Descriptor cost on TRN2 is identical
for 256B and 512B payloads, so pairing halves gather DMA time.  Gathers use
prepare_only + trigger_dma so descriptor generation (Pool) pipelines with
the DMA transfers instead of holding Pool.SEQ through them.

Edges (sorted by dst) are packed densely into 128-slot blocks with per-tile
slot quotas (max over cores) so the block->tile map is identical on every
core.  Aggregation runs per GROUP of TP=2 dst tiles (256 PSUM columns):
per (block, lane, group) one DVE tensor_scalar builds a value-weighted
one-hot S[slot, d] = rs_dst * (dl == iota+off) (4x_2p DVE mode; the rsqrt
deg_d scale rides the one-hot so the epilogue needs no rank-1 bias matmul),
and PE accumulates agg[f, d] += G_lane.T @ S in PSUM.  Per group: one
matmul po[h, d] = W.T @ agg, one ScalarE Lrelu(po + b[h]) (bias per
partition in the [h, d] orientation), fp16 output DMA in the transposed
[h, d] layout (the host de-transposes and converts).
"""

import os as _os
from collections import defaultdict

import numpy as np

SVC_N, INST_N, NODE_N, HID = 20000, 100000, 10000, 128
NCORES = 8
BLK = 128           # slots per block (= PE contraction dim)
LANES = 2           # table rows per gather window (512B / 256B fp16 rows)
TP = 2              # dst tiles per aggregation group (256 PSUM columns)
CHUNK = int(_os.environ.get("GNN_CHUNK", "24"))   # blocks per gather instr
PREP = bool(int(_os.environ.get("GNN_PREP", "0")))  # prepare_only + trigger
OUT_GRP = int(_os.environ.get("GNN_OUT_GRP", "16"))  # dst tiles per out DMA
ACT_MODE = "lrelu"

_cache = {}


def _cdiv(a, b):
    return (a + b - 1) // b


def _rup(a, b):
    return _cdiv(a, b) * b


def _sequence_sources(es, tile):
    """Order this core's used sources so same-tileset sources are adjacent."""
    n = len(es)
    starts = np.flatnonzero(np.r_[True, es[1:] != es[:-1]])
    ends = np.r_[starts[1:], n]
    keys = [tuple(tile[a:b]) for a, b in zip(starts, ends)]
    order = sorted(range(len(starts)), key=lambda i: keys[i])
    return order, starts, ends


def _prep_relation(src, dst, n_src, n_dst, feat_s, rs_d, compact):
    """Host-side sharding/packing for one relation."""
    src = np.asarray(src, np.int64)
    dst = np.asarray(dst, np.int64)

    D = _rup(_cdiv(n_dst, NCORES), 128)  # dst rows per core (padded)
    ntiles = D // 128
    assert ntiles % TP == 0

    cores = []
    for c in range(NCORES):
        lo = c * D
        m = (dst >= lo) & (dst < lo + D)
        es, ed = src[m], dst[m] - lo
        tl = ed >> 7
        order = np.lexsort((tl, es))
        es, ed, tl = es[order], ed[order], tl[order]

        uorder, starts, ends = _sequence_sources(es, tl)
        srcs_u = es[starts]
        nsrc_u = len(srcs_u)

        pos_of_u = np.empty(nsrc_u, np.int64)
        pos_of_u[uorder] = np.arange(nsrc_u)

        if compact:
            table = feat_s[srcs_u[uorder]]
            n_units = nsrc_u
        else:
            used_mask = np.zeros(n_src, bool)
            used_mask[srcs_u] = True
            perm = np.concatenate([srcs_u[uorder],
                                   np.flatnonzero(~used_mask)])
            table = feat_s[perm]
            n_units = n_src

        # slots per tile via the path-greedy pairing over table positions
        slot_k = [[] for _ in range(ntiles)]
        slot_dA = [[] for _ in range(ntiles)]
        slot_dB = [[] for _ in range(ntiles)]
        per_tile = defaultdict(list)  # tile -> list of (pos, [dst_locals])
        for ui in range(nsrc_u):
            a, b = starts[ui], ends[ui]
            p = pos_of_u[ui]
            t0 = a
            while t0 < b:
                t1 = t0
                while t1 < b and tl[t1] == tl[t0]:
                    t1 += 1
                per_tile[tl[t0]].append((p, ed[t0:t1]))
                t0 = t1
        for t, lst in per_tile.items():
            lst.sort(key=lambda x: x[0])
            sk, sa, sb = slot_k[t], slot_dA[t], slot_dB[t]
            prev_pos = -10
            prev_ds = []
            for p, ds in lst:
                ds = list(ds)
                if p == prev_pos + 1 and prev_ds:
                    npair = min(len(prev_ds), len(ds))
                    for i in range(npair):
                        sk.append(prev_pos)
                        sa.append(prev_ds[i])
                        sb.append(ds[i])
                    for d in prev_ds[npair:]:
                        sk.append(prev_pos)
                        sa.append(d)
                        sb.append(-1)
                    ds = ds[npair:]
                else:
                    for d in prev_ds:
                        sk.append(prev_pos)
                        sa.append(d)
                        sb.append(-1)
                prev_pos, prev_ds = p, ds
            for d in prev_ds:
                sk.append(prev_pos)
                sa.append(d)
                sb.append(-1)
            # paired slots first so lane-B tails can be skipped
            osort = sorted(range(len(sk)), key=lambda i: sb[i] < 0)
            slot_k[t] = [sk[i] for i in osort]
            slot_dA[t] = [sa[i] for i in osort]
            slot_dB[t] = [sb[i] for i in osort]

        cores.append(dict(slot_k=slot_k, slot_dA=slot_dA, slot_dB=slot_dB,
                          table=table, n_units=n_units))

    # shared per-tile quotas and block map
    quota = np.zeros(ntiles, np.int64)
    for t in range(ntiles):
        quota[t] = max(max(len(cores[c]["slot_k"][t]) for c in range(NCORES)), 1)
    cum = np.concatenate([[0], np.cumsum(quota)])
    nslot = int(cum[-1])
    nslot_pad = _rup(nslot, BLK)
    nblk = nslot_pad // BLK
    bstart = (cum[:-1] // BLK).astype(np.int64)
    bend = np.minimum(-(-cum[1:] // BLK), nblk).astype(np.int64)
    bend = np.maximum(bend, bstart + 1)
    # T0(b): first tile covering block b; span(b): tiles covered
    T0 = np.zeros(nblk, np.int64)
    cur = 0
    for b in range(nblk):
        while bend[cur] <= b:
            cur += 1
        T0[b] = cur
    span = np.ones(nblk, np.int64)
    for t in range(ntiles):
        for b in range(int(bstart[t]), int(bend[t])):
            span[b] = max(span[b], t - T0[b] + 1)

    # per-core dst rsqrt-degree values (0 beyond n_dst)
    rs_core = []
    for c in range(NCORES):
        lo = c * D
        v = np.zeros(D, np.float32)
        n = max(0, min(D, n_dst - lo))
        if n > 0:
            v[:n] = rs_d[lo:lo + n]
        rs_core.append(v)

    ngrp = ntiles // TP
    activeA = np.zeros((ntiles, nblk), bool)
    activeB = np.zeros((ntiles, nblk), bool)
    for c in range(NCORES):
        d = cores[c]
        kidx = np.zeros(nslot_pad, np.int64)
        dA = np.full(nslot_pad, -1.0, np.float32)
        dB = np.full(nslot_pad, -1.0, np.float32)
        rA = np.zeros(nslot_pad, np.float32)
        rB = np.zeros(nslot_pad, np.float32)
        rsv = rs_core[c]
        for t in range(ntiles):
            off = int(cum[t])
            sk, sa, sb = d["slot_k"][t], d["slot_dA"][t], d["slot_dB"][t]
            for i in range(len(sk)):
                b = (off + i) // BLK
                shift = 128 * int(T0[b])
                kidx[off + i] = sk[i]
                dA[off + i] = sa[i] - shift
                rA[off + i] = rsv[sa[i]]
                activeA[t, b] = True
                if sb[i] >= 0:
                    dB[off + i] = sb[i] - shift
                    rB[off + i] = rsv[sb[i]]
                    activeB[t, b] = True
        # tail pads keep idx 0 (cost model charges num_idxs regardless; a
        # real gather keeps the SBUF block initialized -- NaN x 0 hazard)
        d["kidx"], d["dA"], d["dB"], d["rA"], d["rB"] = kidx, dA, dB, rA, rB
        del d["slot_k"], d["slot_dA"], d["slot_dB"]

    # force one active matmul per tile so every agg gets a start+stop
    for t in range(ntiles):
        if not activeA[t, bstart[t]:bend[t]].any() and \
           not activeB[t, bstart[t]:bend[t]].any():
            activeA[t, bstart[t]] = True

    return dict(cores=cores, ntiles=ntiles, ngrp=ngrp, D=D, n_dst=n_dst,
                nslot=nslot, nslot_pad=nslot_pad, nblk=nblk,
                bstart=bstart, bend=bend, T0=T0, span=span,
                activeA=activeA, activeB=activeB)


def _build_host_data(inputs):
    def prescale(feat, src, n_src):
        deg = np.maximum(np.bincount(np.asarray(src, np.int64),
                                     minlength=n_src), 1.0)
        return (np.asarray(feat, np.float32)
                / np.sqrt(deg)[:, None]).astype(np.float32)

    def rs_of(dstv, n_dst):
        deg = np.maximum(np.bincount(np.asarray(dstv, np.int64),
                                     minlength=n_dst), 1.0)
        return (1.0 / np.sqrt(deg)).astype(np.float32)

    feat0 = prescale(inputs["instance_feat"], inputs["in_src"], INST_N)
    feat1 = prescale(inputs["node_feat"], inputs["ni_src"], NODE_N)
    feat2 = prescale(inputs["svc_feat"], inputs["sc_src"], SVC_N)

    rels = [
        # order matters: output rows are [node_out, inst_out, svc_out]
        _prep_relation(inputs["in_src"], inputs["in_dst"], INST_N, NODE_N,
                       feat0, rs_of(inputs["in_dst"], NODE_N), compact=True),
        _prep_relation(inputs["ni_src"], inputs["ni_dst"], NODE_N, INST_N,
                       feat1, rs_of(inputs["ni_dst"], INST_N), compact=False),
        _prep_relation(inputs["sc_src"], inputs["sc_dst"], SVC_N, SVC_N,
                       feat2, rs_of(inputs["sc_dst"], SVC_N), compact=False),
    ]
    Ws = [inputs["W_inst"], inputs["W_node"], inputs["W_svc"]]
    bs = [inputs["b_inst"], inputs["b_node"], inputs["b_svc"]]

    umax = _rup(max(c["n_units"] for c in rels[0]["cores"]) + 2, 16)
    nblk_tot = sum(r["nblk"] for r in rels)
    nidx_tot = nblk_tot * BLK

    W_cat = np.concatenate([np.asarray(w, np.float32) for w in Ws], axis=1)
    b_col = np.stack([np.asarray(b, np.float32) for b in bs], axis=1)  # [128,3]

    # ramp width: max tile span of any block
    kmax = max(int(r["span"].max()) for r in rels)
    assert kmax * 128 <= 2048, f"ramp {kmax * 128} not fp16-exact"
    iota_ramp = np.tile(np.arange(kmax * 128, dtype=np.float16), (128, 1))

    in_maps = []
    for c in range(NCORES):
        kidx = np.concatenate([r["cores"][c]["kidx"] for r in rels])
        assert kidx.max() < 32768
        idx16 = np.ascontiguousarray(kidx.astype(np.int16).reshape(-1, 16).T)
        idx_sb = np.tile(idx16, (8, 1))

        def blkmaj(name):
            v = np.concatenate([r["cores"][c][name] for r in rels])
            return np.ascontiguousarray(
                v.reshape(nblk_tot, BLK).T).astype(np.float32)

        def mk_tbl(tab, rows):
            out = np.zeros((rows, HID), np.float16)
            out[:len(tab)] = tab.astype(np.float16)
            return np.ascontiguousarray(out)

        in_maps.append({
            "tbl_in": mk_tbl(rels[0]["cores"][c]["table"], umax),
            "tbl_ni": mk_tbl(rels[1]["cores"][c]["table"], NODE_N + 2),
            "tbl_sc": mk_tbl(rels[2]["cores"][c]["table"], SVC_N + 2),
            "idx_sb": np.ascontiguousarray(idx_sb),
            "dA_sb": blkmaj("dA"),
            "dB_sb": blkmaj("dB"),
            "rA_sb": blkmaj("rA"),
            "rB_sb": blkmaj("rB"),
            "W_cat": np.ascontiguousarray(W_cat),
            "b_col": np.ascontiguousarray(b_col),
            "iota_ramp": np.ascontiguousarray(iota_ramp),
        })

    meta = dict(
        umax=umax, nblk_tot=nblk_tot, nidx_tot=nidx_tot, kmax=kmax,
        ntiles=[r["ntiles"] for r in rels],
        ngrps=[r["ngrp"] for r in rels],
        Ds=[r["D"] for r in rels],
        n_dsts=[r["n_dst"] for r in rels],
        nslots=[r["nslot"] for r in rels],
        nblks=[r["nblk"] for r in rels],
        bstarts=[r["bstart"].tolist() for r in rels],
        bends=[r["bend"].tolist() for r in rels],
        T0s=[r["T0"].tolist() for r in rels],
        spans=[r["span"].tolist() for r in rels],
        activeA=[r["activeA"] for r in rels],
        activeB=[r["activeB"] for r in rels],
        tbl_rows=[umax, NODE_N + 2, SVC_N + 2],
    )
    return meta, in_maps


def _build_program(meta):
    import concourse.bacc as bacc
    import concourse.mybir as mybir
    import concourse.tile as tile

    f16 = mybir.dt.float16
    f32 = mybir.dt.float32
    f32r = mybir.dt.float32r
    AF = mybir.ActivationFunctionType
    act_fn = AF.Lrelu if ACT_MODE == "lrelu" else AF.Relu

    nblk_tot, nidx_tot = meta["nblk_tot"], meta["nidx_tot"]
    kmax = meta["kmax"]
    GW = TP * 128  # epilogue group width in dst columns

    nc = bacc.Bacc("TRN2", target_bir_lowering=False, debug=False,
                   enable_asserts=False, num_devices=NCORES)

    tbl_d = [
        nc.dram_tensor(nm, [meta["tbl_rows"][i], HID], f16,
                       kind="ExternalInput")
        for i, nm in enumerate(["tbl_in", "tbl_ni", "tbl_sc"])
    ]
    idx_d = nc.dram_tensor("idx_sb", [128, nidx_tot // 16], mybir.dt.int16,
                           kind="ExternalInput")
    dA_d = nc.dram_tensor("dA_sb", [128, nblk_tot], f32, kind="ExternalInput")
    dB_d = nc.dram_tensor("dB_sb", [128, nblk_tot], f32, kind="ExternalInput")
    rA_d = nc.dram_tensor("rA_sb", [128, nblk_tot], f32, kind="ExternalInput")
    rB_d = nc.dram_tensor("rB_sb", [128, nblk_tot], f32, kind="ExternalInput")
    W_d = nc.dram_tensor("W_cat", [128, 3 * HID], f32, kind="ExternalInput")
    b_d = nc.dram_tensor("b_col", [128, 3], f32, kind="ExternalInput")
    ior_d = nc.dram_tensor("iota_ramp", [128, kmax * 128], f16,
                           kind="ExternalInput")

    out_d = [
        nc.dram_tensor(nm, [128, meta["ntiles"][i] * 128], f16,
                       kind="ExternalOutput")
        for i, nm in enumerate(["out_node", "out_inst", "out_svc"])
    ]

    with tile.TileContext(nc) as tc:
        with (
            tc.tile_pool(name="const", bufs=1) as const,
            tc.tile_pool(name="g", bufs=6) as gpool,
            tc.tile_pool(name="st", bufs=12) as stpool,
            tc.tile_pool(name="evac", bufs=4) as evac,
            tc.tile_pool(name="osb", bufs=4) as opool,
            tc.tile_pool(name="psA", bufs=6, space="PSUM") as psA,
            tc.tile_pool(name="psO", bufs=2, space="PSUM") as psO,
        ):
            # load the leading idx slice first so gathers start ASAP
            idx_t = const.tile([128, nidx_tot // 16], mybir.dt.int16)
            c0 = min(2 * CHUNK * BLK // 16, nidx_tot // 16)
            nc.sync.dma_start(idx_t[:, :c0], idx_d.ap()[:, :c0])
            dA_t = const.tile([128, nblk_tot], f32)
            nc.sync.dma_start(dA_t[:], dA_d.ap())
            dB_t = const.tile([128, nblk_tot], f32)
            nc.sync.dma_start(dB_t[:], dB_d.ap())
            rA_t = const.tile([128, nblk_tot], f32)
            nc.sync.dma_start(rA_t[:], rA_d.ap())
            rB_t = const.tile([128, nblk_tot], f32)
            nc.sync.dma_start(rB_t[:], rB_d.ap())
            ior_t = const.tile([128, kmax * 128], f16)
            nc.sync.dma_start(ior_t[:], ior_d.ap())
            W_t = const.tile([128, 3 * HID], f32)
            nc.sync.dma_start(W_t[:], W_d.ap())
            W_r = const.tile([128, 3 * HID], f32r)
            nc.scalar.copy(W_r[:], W_t[:])
            b_t = const.tile([128, 3], f32)
            nc.sync.dma_start(b_t[:], b_d.ap())
            if c0 < nidx_tot // 16:
                nc.sync.dma_start(idx_t[:, c0:], idx_d.ap()[:, c0:])

            dma_sems = {}
            g_tiles = {}    # global chunk -> gather tile
            st_tiles = {}   # (block, lane, kg) -> one-hot [128, GW]

            def issue_gather(ci, rel, rel_cblk0, rel_blk0, rel_nblk, rel_nslot):
                local_b0 = (ci - rel_cblk0) * CHUNK
                cblk = min(CHUNK, rel_nblk - local_b0)
                gt = gpool.tile([128, CHUNK, LANES * HID], f16, tag="g")
                nidx = cblk * BLK
                reg = nidx
                off16 = (rel_blk0 + local_b0) * BLK // 16
                in_ap = tbl_d[rel].ap()
                in_ap.ap[0] = [HID, meta["tbl_rows"][rel] - 1]
                in_ap.ap[1] = [1, LANES * HID]
                if PREP:
                    dma_sems[ci] = nc.alloc_semaphore(f"gdma{ci}")
                kw = dict(prepare_only=True, sem=dma_sems[ci]) if PREP else {}
                nc.gpsimd.dma_gather(
                    out_ap=gt[:, :cblk, :],
                    in_ap=in_ap,
                    idxs_ap=idx_t[:, off16:off16 + nidx // 16],
                    num_idxs=nidx,
                    num_idxs_reg=reg,
                    elem_size=LANES * HID,
                    elem_step=HID,
                    single_packet=False,
                    **kw,
                )
                if PREP:
                    nc.gpsimd.trigger_dma(count=1)
                g_tiles[ci] = gt

            def issue_st(gb, lane, wid, dl_t, rs_t):
                # value-weighted one-hot: rs_dst * (dl == iota), one DVE op in
                # 4x_2p mode (fp16 packed in/out; f32 scalar APs are exempt).
                # Built once per (block, lane) covering the block's full tile
                # span; per-tile matmuls slice 128-column windows from it.
                st = stpool.tile([128, kmax * 128], f16, tag="st")
                nc.vector.tensor_scalar(
                    st[:, :wid], ior_t[:, :wid],
                    dl_t[:, gb:gb + 1], rs_t[:, gb:gb + 1],
                    mybir.AluOpType.is_equal, mybir.AluOpType.mult)
                st_tiles[(gb, lane)] = st

            cblk_base = 0
            blk_base = 0
            for rel in range(3):
                ngrp = meta["ngrps"][rel]
                nblk = meta["nblks"][rel]
                nslot = meta["nslots"][rel]
                bstart = meta["bstarts"][rel]
                bend = meta["bends"][rel]
                T0 = meta["T0s"][rel]
                span = meta["spans"][rel]
                actA = meta["activeA"][rel]
                actB = meta["activeB"][rel]
                for g in range(ngrp):
                    aggs = []
                    for kt in range(TP):
                        t = g * TP + kt
                        ems = []
                        for b in range(int(bstart[t]), int(bend[t])):
                            if actA[t, b]:
                                ems.append((b, 0))
                            if actB[t, b]:
                                ems.append((b, 1))
                        agg = psA.tile([128, 128], f32, tag="agg")
                        for i, (b, lane) in enumerate(ems):
                            gb = blk_base + b
                            ci = cblk_base + b // CHUNK
                            if ci not in g_tiles:
                                issue_gather(ci, rel, cblk_base, blk_base,
                                             nblk, nslot)
                            if (gb, lane) not in st_tiles:
                                issue_st(gb, lane, int(span[b]) * 128,
                                         dA_t if lane == 0 else dB_t,
                                         rA_t if lane == 0 else rB_t)
                            k = t - int(T0[b])
                            cj = b % CHUNK
                            nc.tensor.matmul(
                                agg[:],
                                g_tiles[ci][:, cj, lane * HID:(lane + 1) * HID],
                                st_tiles[(gb, lane)][:, k * 128:(k + 1) * 128],
                                start=(i == 0), stop=(i == len(ems) - 1))
                        aggs.append(agg)
                    # epilogue: po[h, d] = W.T @ agg_pair; Lrelu(po + b[h])
                    aggsb = evac.tile([128, GW], f32r, tag="evac")
                    for kt in range(TP):
                        nc.scalar.copy(aggsb[:, kt * 128:(kt + 1) * 128],
                                       aggs[kt][:])
                    po = psO.tile([128, GW], f32, tag="po")
                    nc.tensor.matmul(
                        po[:],
                        W_r[:, rel * HID:(rel + 1) * HID],
                        aggsb[:],
                        start=True, stop=True)
                    og = g % (OUT_GRP // TP)
                    if og == 0:
                        osb = opool.tile([128, OUT_GRP * 128], f16, tag="osb")
                        osb_g0 = g
                    nc.scalar.activation(
                        osb[:, og * GW:(og + 1) * GW], po[:], act_fn,
                        bias=b_t[:, rel:rel + 1], scale=1.0, alpha=0.01)
                    if og == OUT_GRP // TP - 1 or g == ngrp - 1:
                        cols = (g - osb_g0 + 1) * GW
                        dst = out_d[rel].ap()[:, osb_g0 * GW:
                                              osb_g0 * GW + cols]
                        nc.sync.dma_start(dst, osb[:, :cols])
                cblk_base += _cdiv(nblk, CHUNK)
                blk_base += nblk

    nc.compile()
    return nc


def _run(nc, in_maps, trace=False, **kw):
    from concourse import bass_utils
    res = bass_utils.run_bass_kernel_spmd(
        nc, in_maps, core_ids=list(range(NCORES)), trace=trace, **kw)
    return res


def _assemble(results, meta):
    out = np.empty((NODE_N + INST_N + SVC_N, HID), np.float32)
    offs = [0, NODE_N, NODE_N + INST_N]
    names = ["out_node", "out_inst", "out_svc"]
    for rel in range(3):
        D, n_dst = meta["Ds"][rel], meta["n_dsts"][rel]
        ntiles = meta["ntiles"][rel]
        for c in range(NCORES):
            lo = c * D
            n = max(0, min(D, n_dst - lo))
            if n > 0:
                arr = results[c][names[rel]]  # [128 h, ntiles*128 d] fp16
                rows = np.ascontiguousarray(
                    arr.reshape(128, ntiles, 128).transpose(1, 2, 0)
                ).reshape(-1, HID)[:n].astype(np.float32)
                out[offs[rel] + lo: offs[rel] + lo + n] = rows
    return out


def kernel(**inputs):
    import hashlib
    key = "prog"
    h = hashlib.sha1()
    for k in ("sc_src", "sc_dst", "in_src", "in_dst", "ni_src", "ni_dst"):
        h.update(np.ascontiguousarray(np.asarray(inputs[k], np.int32)).tobytes())
    sig = h.hexdigest()
    meta, in_maps = _build_host_data(inputs)
    if key in _cache and _cache[key][0] == sig:
        _, nc, _ = _cache[key]
    else:
        nc = _build_program(meta)
        _cache[key] = (sig, nc, meta)
    res = _run(nc, in_maps)
    return _assemble(res.results, meta)
